# revision 7
# baseline (speedup 1.0000x reference)
"""Trainium2 Bass kernel for the quantized (I-BERT style) ViT block.

kernel(**inputs) takes the FULL unsharded inputs (as in setup_inputs()) and
returns the FULL output matching reference(**inputs) -> (x, scaling_factor).

Data-parallel over batch across 8 NeuronCores (8 images / core). Per-tensor
activation scale factors use a scalar AllReduce(max) at each quantization
point. Stage outputs that must wait for a global max are spilled to DRAM and
re-streamed for the quantize pass (SBUF cannot hold them all).

Degenerate path note: the reference's integer softmax collapses for any
realistic data (exp_sum >> 2^32 so factor=0), making attn@v exactly 0; the
reference's quant_act then computes 0/0 = NaN and the output is all-NaN.  The
device computes the same pipeline with guarded reciprocals (hardware clamps
instead of producing IEEE NaN); the host applies the reference's exact
semantics when the device-computed global max is 0.
"""

import math

import numpy as np
import ml_dtypes

B, N, C, HID, H = 64, 197, 768, 3072, 12
D = C // H
NCORES = 8
PER = B // NCORES
M = PER * N                # 1576 tokens per core
PT = 128
NT = (M + PT - 1) // PT    # 13
CT = C // PT               # 6
OT_QKV = 3 * C // PT       # 18
OT_FC1 = HID // PT         # 24
CHUNK = 512
CHUNKS = [(i, min(CHUNK, M - i)) for i in range(0, M, CHUNK)]


def _f32(x):
    return np.float32(x)


def _ternarize(w):
    w = np.asarray(w, np.float32)
    thr = _f32(0.7) * _f32(np.mean(np.abs(w)))
    mask = (np.abs(w) > thr).astype(np.float32)
    alpha = _f32(np.sum(np.abs(w) * mask) / max(np.sum(mask), 1.0))
    return np.sign(w).astype(np.float32) * mask, alpha


def _quant_w(w):
    w = np.asarray(w, np.float32)
    w_sf = np.max(np.abs(w), axis=1).astype(np.float32) / _f32(127.0)
    w_int = np.float32(np.round(w / w_sf[:, None]))
    return w_int, w_sf


_BUILT = None


def _build_module():
    import concourse.bass as bass
    import concourse.mybir as mybir
    import concourse.tile as tile
    from concourse import bacc
    from concourse.masks import make_identity

    dt = mybir.dt
    op = mybir.AluOpType
    act = mybir.ActivationFunctionType
    AX = mybir.AxisListType

    nc = bacc.Bacc("TRN2", target_bir_lowering=False, debug=False)

    x1_d = nc.dram_tensor("x1", [M, C], dt.float32, kind="ExternalInput")
    wqkvT_d = nc.dram_tensor("wqkvT", [C, 3 * C], dt.bfloat16, kind="ExternalInput")
    wprojT_d = nc.dram_tensor("wprojT", [C, C], dt.bfloat16, kind="ExternalInput")
    wfc1T_d = nc.dram_tensor("wfc1T", [C, HID], dt.bfloat16, kind="ExternalInput")
    wfc2T_d = nc.dram_tensor("wfc2T", [HID, C], dt.bfloat16, kind="ExternalInput")
    ln1bi_d = nc.dram_tensor("ln1bi", [1, C], dt.float32, kind="ExternalInput")
    ln1sc_d = nc.dram_tensor("ln1sc", [1, C], dt.float32, kind="ExternalInput")
    ln2bi_d = nc.dram_tensor("ln2bi", [1, C], dt.float32, kind="ExternalInput")
    ln2sc_d = nc.dram_tensor("ln2sc", [1, C], dt.float32, kind="ExternalInput")
    bpw_d = nc.dram_tensor("bpw", [1, C], dt.float32, kind="ExternalInput")
    wsf_d = nc.dram_tensor("wsf", [1, C], dt.float32, kind="ExternalInput")
    bfc1_d = nc.dram_tensor("bfc1", [1, HID], dt.float32, kind="ExternalInput")
    bfc2_d = nc.dram_tensor("bfc2", [1, C], dt.float32, kind="ExternalInput")
    scal_d = nc.dram_tensor("scal", [1, 8], dt.float32, kind="ExternalInput")
    out_d = nc.dram_tensor("out", [M, C], dt.float32, kind="ExternalOutput")
    stats_d = nc.dram_tensor("stats", [1, 16], dt.float32, kind="ExternalOutput")

    with tile.TileContext(nc) as tc:
        import contextlib
        ctx = contextlib.ExitStack()
        with ctx:
            sing = ctx.enter_context(tc.tile_pool(name="sing", bufs=1))
            scp = ctx.enter_context(tc.tile_pool(name="scp", bufs=1))
            dr = ctx.enter_context(tc.tile_pool(name="dr", bufs=1, space="DRAM"))
            tp = ctx.enter_context(tc.tile_pool(name="tp", bufs=2))
            wp = ctx.enter_context(tc.tile_pool(name="wp", bufs=8))
            wp2 = ctx.enter_context(tc.tile_pool(name="wp2", bufs=26))
            pp = ctx.enter_context(tc.tile_pool(name="pp", bufs=2, space="PSUM"))
            pt2 = ctx.enter_context(tc.tile_pool(name="pt2", bufs=1, space="PSUM"))

            idf = sing.tile([128, 128], dt.float32, name="idf")
            make_identity(nc, idf)
            idb = sing.tile([128, 128], dt.bfloat16, name="idb")
            make_identity(nc, idb)

            def bcast_dram(dten, name, offset=0):
                b = scp.tile([128, 1], dt.float32, name=f"b_{name}", tag=f"b_{name}")
                nc.sync.dma_start(
                    out=b, in_=bass.AP(tensor=dten, offset=offset, ap=[[0, 128], [1, 1]])
                )
                return b

            def bcast(src_ap, name):
                d = dr.tile([1, 1], dt.float32, name=f"d_{name}", tag=f"d_{name}")
                nc.sync.dma_start(out=d, in_=src_ap)
                return bcast_dram(d.tensor, name, d.offset)

            cc_n = [0]

            def allreduce_max(local_ap, name):
                cc_n[0] += 1
                i = cc_n[0]
                di = dr.tile([1, 1], dt.float32, name=f"cci_{i}", tag=f"cci_{i}")
                do = dr.tile([1, 1], dt.float32, name=f"cco_{i}", tag=f"cco_{i}",
                             addr_space="Shared")
                nc.sync.dma_start(out=di, in_=local_ap)
                nc.gpsimd.collective_compute(
                    "AllReduce", op.max, replica_groups=[list(range(NCORES))],
                    ins=[di.opt()], outs=[do.opt()],
                )
                g = scp.tile([1, 1], dt.float32, name=f"ccg_{i}", tag=f"ccg_{i}")
                nc.sync.dma_start(out=g, in_=do)
                return g

            sc_i = [0]

            def sc_tile(n=1):
                sc_i[0] += 1
                return scp.tile([1, n], dt.float32, name=f"s{sc_i[0]}", tag=f"s{sc_i[0]}")

            def sc_op(a_ap, alu, s1, s2=None, alu2=None):
                o = sc_tile()
                nc.vector.tensor_scalar(out=o, in0=a_ap, scalar1=s1, scalar2=s2, op0=alu,
                                        **({"op1": alu2} if alu2 is not None else {}))
                return o

            def sc_recip(a_ap):
                o = sc_tile()
                nc.vector.reciprocal(out=o, in_=a_ap)
                return o

            def sc_mul2(a_ap, b_ap):
                o = sc_tile()
                nc.vector.tensor_tensor(out=o, in0=a_ap, in1=b_ap, op=op.mult)
                return o

            def sc_floor(a_ap):
                sc_i[0] += 1
                i = scp.tile([1, 1], dt.int32, name=f"fi{sc_i[0]}", tag=f"fi{sc_i[0]}")
                nc.vector.tensor_copy(out=i, in_=a_ap)
                f = sc_tile()
                nc.vector.tensor_copy(out=f, in_=i)
                g = sc_tile()
                nc.vector.tensor_tensor(out=g, in0=f, in1=a_ap, op=op.is_gt)
                o = sc_tile()
                nc.vector.tensor_tensor(out=o, in0=f, in1=g, op=op.subtract)
                return o

            class MaxAcc:
                def __init__(self, name):
                    self.t = scp.tile([128, 1], dt.float32, name=f"mx_{name}", tag=f"mx_{name}")
                    nc.vector.memset(self.t, 0.0)
                    self.name = name

                def add(self, red_ap, p0=0):
                    p = red_ap.partition_size()
                    nc.vector.tensor_tensor(out=self.t[p0:p0 + p], in0=self.t[p0:p0 + p],
                                            in1=red_ap, op=op.max)

                def add_from(self, src_ap, scratch):
                    nc.vector.tensor_reduce(out=scratch, in_=src_ap, axis=AX.X,
                                            op=op.max, apply_absolute_value=True)
                    self.add(scratch)

                def finish(self):
                    o = scp.tile([1, 1], dt.float32, name=f"mg_{self.name}", tag=f"mg_{self.name}")
                    nc.gpsimd.tensor_reduce(out=o, in_=self.t, axis=AX.C, op=op.max)
                    return o

            scals = sing.tile([1, 8], dt.float32, name="scals")
            nc.sync.dma_start(out=scals, in_=scal_d[:])

            def load_vec_b(dten, n, name):
                t = sing.tile([128, n], dt.float32, name=name)
                nc.sync.dma_start(out=t, in_=bass.AP(tensor=dten, offset=0, ap=[[0, 128], [1, n]]))
                return t

            ln1bi = load_vec_b(ln1bi_d, C, "ln1bi_t")
            ln1sc = load_vec_b(ln1sc_d, C, "ln1sc_t")
            ln2bi = load_vec_b(ln2bi_d, C, "ln2bi_t")
            ln2sc = load_vec_b(ln2sc_d, C, "ln2sc_t")

            def load_cm(dten, nt_, name):
                # [1, nt_*128] channel-major -> [128, nt_] (partition = channel % 128)
                t = sing.tile([128, nt_], dt.float32, name=name)
                nc.sync.dma_start(out=t, in_=bass.AP(tensor=dten, offset=0,
                                                     ap=[[1, 128], [128, nt_]]))
                return t

            wsf_cm = load_cm(wsf_d, CT, "wsf_cm")
            bpw_cm = load_cm(bpw_d, CT, "bpw_cm")
            bfc1_cm = load_cm(bfc1_d, OT_FC1, "bfc1_cm")
            bfc2_cm = load_cm(bfc2_d, CT, "bfc2_cm")

            def floor_cm(src, nt_, rcp_b, name):
                # floor(src * rcp) exact, per-channel [128, nt_]
                x = sing.tile([128, nt_], dt.float32, name=f"{name}_x")
                nc.vector.tensor_scalar(out=x, in0=src, scalar1=rcp_b[:, 0:1], scalar2=None,
                                        op0=op.mult)
                i = sing.tile([128, nt_], dt.int32, name=f"{name}_i")
                nc.vector.tensor_copy(out=i, in_=x)
                f = sing.tile([128, nt_], dt.float32, name=f"{name}_f")
                nc.vector.tensor_copy(out=f, in_=i)
                g = sing.tile([128, nt_], dt.float32, name=f"{name}_g")
                nc.vector.tensor_tensor(out=g, in0=f, in1=x, op=op.is_gt)
                nc.vector.tensor_tensor(out=f, in0=f, in1=g, op=op.subtract)
                return f

            # ---------------- LN stage (token-major, streaming from DRAM) ----------
            def ln_stage(src_d, bias_t, sfc_t, rcp_in_b, tag):
                spill = dr.tile([M, C], dt.float32, name=f"lnsp_{tag}", tag=f"lnsp_{tag}")
                mx = MaxAcc(f"ln_{tag}")
                for it in range(NT):
                    p = min(PT, M - it * PT)
                    xin = tp.tile([128, C], dt.float32, name=f"xin_{tag}_{it}", tag="ln_xin")
                    nc.sync.dma_start(out=xin[:p, :], in_=src_d[it * PT:it * PT + p, :])
                    xint = tp.tile([128, C], dt.float32, name=f"xi_{tag}_{it}", tag="ln_xi")
                    rs = tp.tile([128, 1], dt.float32, name=f"rs_{tag}_{it}", tag="ln_rs")
                    if rcp_in_b is not None:
                        nc.scalar.activation(out=xint[:p, :], in_=xin[:p, :], func=act.Copy,
                                             scale=rcp_in_b[:p, 0:1], accum_out=rs[:p, 0:1])
                    else:
                        nc.scalar.activation(out=xint[:p, :], in_=xin[:p, :], func=act.Copy,
                                             scale=1.0, accum_out=rs[:p, 0:1])
                    mi = tp.tile([128, 1], dt.int32, name=f"mi_{tag}_{it}", tag="ln_mi")
                    mf = tp.tile([128, 1], dt.float32, name=f"mf_{tag}_{it}", tag="ln_mf")
                    nc.vector.tensor_scalar(out=mf[:p], in0=rs[:p], scalar1=float(_f32(1.0 / C)),
                                            scalar2=None, op0=op.mult)
                    nc.vector.tensor_copy(out=mi[:p], in_=mf[:p])
                    nc.vector.tensor_copy(out=mf[:p], in_=mi[:p])
                    y = tp.tile([128, C], dt.float32, name=f"y_{tag}_{it}", tag="ln_y")
                    nc.vector.tensor_scalar(out=y[:p, :], in0=xint[:p, :], scalar1=mf[:p, 0:1],
                                            scalar2=None, op0=op.subtract)
                    sq = tp.tile([128, C], dt.float32, name=f"sq_{tag}_{it}", tag="ln_sq")
                    var = tp.tile([128, 1], dt.float32, name=f"v_{tag}_{it}", tag="ln_v")
                    nc.scalar.activation(out=sq[:p, :], in_=y[:p, :], func=act.Square,
                                         accum_out=var[:p, 0:1])
                    nc.vector.tensor_scalar(out=var[:p], in0=var[:p], scalar1=1.0, scalar2=None,
                                            op0=op.max)
                    std = tp.tile([128, 1], dt.float32, name=f"st_{tag}_{it}", tag="ln_st")
                    nc.scalar.activation(out=std[:p], in_=var[:p], func=act.Sqrt)
                    sti = tp.tile([128, 1], dt.int32, name=f"sti_{tag}_{it}", tag="ln_sti")
                    nc.vector.tensor_scalar(out=std[:p], in0=std[:p], scalar1=0.4999999,
                                            scalar2=None, op0=op.subtract)
                    nc.vector.tensor_copy(out=sti[:p], in_=std[:p])
                    nc.vector.tensor_copy(out=std[:p], in_=sti[:p])
                    rstd = tp.tile([128, 1], dt.float32, name=f"rst_{tag}_{it}", tag="ln_rst")
                    nc.vector.reciprocal(out=rstd[:p], in_=std[:p])
                    fac = tp.tile([128, 1], dt.float32, name=f"fa_{tag}_{it}", tag="ln_fa")
                    nc.vector.tensor_scalar(out=fac[:p], in0=rstd[:p], scalar1=float(2.0 ** 31),
                                            scalar2=0.49, op0=op.mult, op1=op.subtract)
                    fai = tp.tile([128, 1], dt.int32, name=f"fai_{tag}_{it}", tag="ln_fai")
                    nc.vector.tensor_copy(out=fai[:p], in_=fac[:p])
                    nc.vector.tensor_copy(out=fac[:p], in_=fai[:p])
                    nc.vector.tensor_scalar(out=fac[:p], in0=fac[:p], scalar1=0.5, scalar2=None,
                                            op0=op.mult)
                    # floor(y*factor/2) = castRNE(y*(factor/2) - 0.25): args are ints/half-ints
                    nc.vector.tensor_scalar(out=y[:p, :], in0=y[:p, :], scalar1=fac[:p, 0:1],
                                            scalar2=0.25, op0=op.mult, op1=op.subtract)
                    yi = tp.tile([128, C], dt.int32, name=f"yi_{tag}_{it}", tag="ln_yi")
                    nc.vector.tensor_copy(out=yi[:p, :], in_=y[:p, :])
                    nc.vector.tensor_copy(out=y[:p, :], in_=yi[:p, :])
                    nc.vector.tensor_tensor(out=y[:p, :], in0=y[:p, :], in1=bias_t[:p, :],
                                            op=op.add)
                    nc.vector.tensor_tensor(out=y[:p, :], in0=y[:p, :], in1=sfc_t[:p, :],
                                            op=op.mult)
                    red = tp.tile([128, 1], dt.float32, name=f"re_{tag}_{it}", tag="ln_re")
                    mx.add_from(y[:p, :], red[:p])
                    nc.sync.dma_start(out=spill[it * PT:it * PT + p, :], in_=y[:p, :])
                return spill, mx.finish()

            def quant_transpose(spill, rcp_b, dst_tiles, tag):
                """spill [M,C] f32 -> round -> bf16 -> PE transpose -> dst [CT][128, M]"""
                for it in range(NT):
                    p = min(PT, M - it * PT)
                    yb = tp.tile([128, C], dt.float32, name=f"qy_{tag}_{it}", tag="qt_y")
                    nc.sync.dma_start(out=yb[:p, :], in_=spill[it * PT:it * PT + p, :])
                    qi = tp.tile([128, C], dt.int32, name=f"q_{tag}_{it}", tag="qt_qi")
                    nc.scalar.activation(out=qi[:p, :], in_=yb[:p, :], func=act.Copy,
                                         scale=rcp_b[:p, 0:1])
                    qb = tp.tile([128, C], dt.bfloat16, name=f"qb_{tag}_{it}", tag="qt_qb")
                    nc.vector.tensor_copy(out=qb[:p, :], in_=qi[:p, :])
                    for c in range(CT):
                        ps = pt2.tile([128, 128], dt.bfloat16, name=f"pt_{tag}_{it}_{c}",
                                      tag="qt_ps")
                        nc.tensor.transpose(ps[:, :], qb[:, c * PT:(c + 1) * PT], idb[:])
                        nc.scalar.activation(out=dst_tiles[c][:, it * PT:it * PT + p],
                                             in_=ps[:, :p], func=act.Copy)

            # ========================= LN1 + qa1 =========================
            ln1_spill, ln1_lmax = ln_stage(x1_d, ln1bi, ln1sc,
                                           bcast_dram(scal_d, "rcpsf1", 0), "l1")
            g1 = allreduce_max(ln1_lmax, "qa1")
            sf_qa1 = sc_op(g1, op.mult, float(_f32(1.0 / 127.0)))
            sf_qa1_g = sc_op(sf_qa1, op.max, 1e-37)
            rcp_qa1_b = bcast(sc_recip(sf_qa1_g), "rq1")

            with tc.tile_pool(name="mats1", bufs=1) as mats1:
                xqT = [mats1.tile([128, M], dt.bfloat16, name=f"xqT_{c}", tag=f"xqT_{c}")
                       for c in range(CT)]
                quant_transpose(ln1_spill, rcp_qa1_b, xqT, "x1")

                # ===================== QKV matmul -> spill =====================
                qkv_spill = dr.tile([3 * C, M], dt.float32, name="qkv_spill")
                mx_qkv = MaxAcc("qkv")
                for ot in range(OT_QKV):
                    wts = []
                    for kt in range(CT):
                        w = wp.tile([128, 128], dt.bfloat16, name=f"wq_{ot}_{kt}", tag="wq")
                        nc.sync.dma_start(out=w, in_=wqkvT_d[kt * PT:(kt + 1) * PT,
                                                            ot * PT:(ot + 1) * PT])
                        wts.append(w)
                    for (c0, cw) in CHUNKS:
                        ps = pp.tile([128, CHUNK], dt.float32, name=f"pq_{ot}_{c0}", tag="pq")
                        for kt in range(CT):
                            nc.tensor.matmul(ps[:, :cw], wts[kt][:, :], xqT[kt][:, c0:c0 + cw],
                                             start=(kt == 0), stop=(kt == CT - 1))
                        sb = tp.tile([128, CHUNK], dt.float32, name=f"sq_{ot}_{c0}", tag="mm_sb")
                        nc.scalar.activation(out=sb[:, :cw], in_=ps[:, :cw], func=act.Copy)
                        red = tp.tile([128, 1], dt.float32, name=f"rq_{ot}_{c0}", tag="mm_red")
                        mx_qkv.add_from(sb[:, :cw], red)
                        nc.sync.dma_start(out=qkv_spill[ot * PT:(ot + 1) * PT, c0:c0 + cw],
                                          in_=sb[:, :cw])
                qkv_lmax = mx_qkv.finish()
            g2 = allreduce_max(qkv_lmax, "qa2")
            out_sf_qkv = sc_mul2(sf_qa1, scals[0:1, 1:2])
            sf1a = sc_op(sc_mul2(g2, out_sf_qkv), op.mult, float(_f32(1.0 / 127.0)))
            sf1a_g = sc_op(sf1a, op.max, 1e-37)
            ratio_qkv_b = bcast(sc_mul2(out_sf_qkv, sc_recip(sf1a_g)), "rqkv")

            with tc.tile_pool(name="qkp", bufs=1) as qkp:
                qkvT = [qkp.tile([128, M], dt.bfloat16, name=f"qkvT_{t}", tag=f"qkvT_{t}")
                        for t in range(OT_QKV)]
                for ot in range(OT_QKV):
                    for (c0, cw) in CHUNKS:
                        sb = tp.tile([128, CHUNK], dt.float32, name=f"uq_{ot}_{c0}", tag="mm_u")
                        nc.sync.dma_start(out=sb[:, :cw],
                                          in_=qkv_spill[ot * PT:(ot + 1) * PT, c0:c0 + cw])
                        qi = tp.tile([128, CHUNK], dt.int32, name=f"uqi_{ot}_{c0}", tag="mm_ui")
                        nc.scalar.activation(out=qi[:, :cw], in_=sb[:, :cw], func=act.Copy,
                                             scale=ratio_qkv_b[:, 0:1])
                        nc.vector.tensor_copy(out=qkvT[ot][:, c0:c0 + cw], in_=qi[:, :cw])

                def qT_ap(h, b):
                    return qkvT[h // 2][(h % 2) * 64:(h % 2) * 64 + 64, b * N:(b + 1) * N]

                def kT_ap(h, b):
                    return qkvT[CT + h // 2][(h % 2) * 64:(h % 2) * 64 + 64, b * N:(b + 1) * N]

                def vT_ap(h, b):
                    return qkvT[2 * CT + h // 2][(h % 2) * 64:(h % 2) * 64 + 64,
                                                 b * N:(b + 1) * N]

                NS = [(0, 128), (128, 69)]
                # ----- scores pass 1: absmax only -----
                mx_s = MaxAcc("scores")
                for b in range(PER):
                    for h in range(H):
                        for (n0, nw) in NS:
                            ps = pt2.tile([128, N], dt.float32, name=f"ps_{b}_{h}_{n0}",
                                          tag="ps_s")
                            nc.tensor.matmul(ps[:nw, :], qT_ap(h, b)[:, n0:n0 + nw], kT_ap(h, b),
                                             start=True, stop=True)
                            red = tp.tile([128, 1], dt.float32, name=f"rs_{b}_{h}_{n0}",
                                          tag="mm_red")
                            mx_s.add_from(ps[:nw, :], red[:nw])
                s_lmax = mx_s.finish()
                g3 = allreduce_max(s_lmax, "qa3")
                sfa = sc_op(sc_mul2(sf1a, sf1a), op.mult, float(_f32(D ** -0.5)))
                sf_s = sc_op(sc_mul2(g3, sfa), op.mult, float(_f32(1.0 / 127.0)))
                sf_s_g = sc_op(sf_s, op.max, 1e-37)
                rcp_sf_s = sc_recip(sf_s_g)
                ratio_s_b = bcast(sc_mul2(sfa, rcp_sf_s), "rs")
                x0i = sc_floor(sc_op(rcp_sf_s, op.mult, -0.6931))
                bi_s = sc_floor(sc_op(rcp_sf_s, op.mult, float(_f32(0.96963238 / 0.35815147))))
                ci_s = sc_floor(sc_op(sc_mul2(rcp_sf_s, rcp_sf_s), op.mult,
                                      float(_f32(1.0 / 0.35815147))))
                clamp_b = bcast(sc_op(x0i, op.mult, 30.0), "clmp")
                rcpx0_b = bcast(sc_recip(x0i), "rcpx0")
                negx0_b = bcast(sc_op(x0i, op.mult, -1.0), "negx0")
                bi_b = bcast(bi_s, "bis")
                ci_b = bcast(ci_s, "cis")

                av_spill = dr.tile([C, M], dt.float32, name="av_spill")
                mx_av = MaxAcc("av")
                with tc.tile_pool(name="smp", bufs=2) as smp:
                    for b in range(PER):
                        for h in range(H):
                            r0v = (h % 2) * 64
                            vtok = []
                            for (n0, nw) in NS:
                                pv = pt2.tile([128, 64], dt.bfloat16, name=f"pv_{b}_{h}_{n0}",
                                              tag="pv")
                                nc.tensor.transpose(pv[:nw, :], vT_ap(h, b)[:, n0:n0 + nw],
                                                    idb[r0v:r0v + 64, r0v:r0v + 64])
                                vt = smp.tile([128, 64], dt.float32, name=f"vt_{b}_{h}_{n0}",
                                              tag=f"vt_{n0}")
                                nc.scalar.activation(out=vt[:nw, :], in_=pv[:nw, :],
                                                     func=act.Copy)
                                vtok.append(vt)
                            at_parts = {}
                            for (n0, nw) in NS:
                                ps = pt2.tile([128, N], dt.float32, name=f"p2_{b}_{h}_{n0}",
                                              tag="ps_s")
                                nc.tensor.matmul(ps[:nw, :], qT_ap(h, b)[:, n0:n0 + nw],
                                                 kT_ap(h, b), start=True, stop=True)
                                xi = smp.tile([128, N], dt.int32, name=f"sxi_{b}_{h}_{n0}",
                                              tag="sm_xi")
                                nc.scalar.activation(out=xi[:nw, :], in_=ps[:nw, :],
                                                     func=act.Copy, scale=ratio_s_b[:nw, 0:1])
                                x = smp.tile([128, N], dt.float32, name=f"sx_{b}_{h}_{n0}",
                                             tag="sm_x")
                                nc.vector.tensor_copy(out=x[:nw, :], in_=xi[:nw, :])
                                rm = smp.tile([128, 1], dt.float32, name=f"srm_{b}_{h}_{n0}",
                                              tag="sm_rm")
                                nc.vector.tensor_reduce(out=rm[:nw], in_=x[:nw, :], axis=AX.X,
                                                        op=op.max)
                                nc.vector.tensor_scalar(out=x[:nw, :], in0=x[:nw, :],
                                                        scalar1=rm[:nw, 0:1],
                                                        scalar2=clamp_b[:nw, 0:1],
                                                        op0=op.subtract, op1=op.max)
                                qf = smp.tile([128, N], dt.float32, name=f"sqf_{b}_{h}_{n0}",
                                              tag="sm_qf")
                                nc.vector.tensor_scalar(out=qf[:nw, :], in0=x[:nw, :],
                                                        scalar1=rcpx0_b[:nw, 0:1], scalar2=0.49,
                                                        op0=op.mult, op1=op.subtract)
                                qi32 = smp.tile([128, N], dt.int32, name=f"sqi_{b}_{h}_{n0}",
                                                tag="sm_qi")
                                nc.vector.tensor_copy(out=qi32[:nw, :], in_=qf[:nw, :])
                                nc.vector.tensor_copy(out=qf[:nw, :], in_=qi32[:nw, :])
                                r = smp.tile([128, N], dt.float32, name=f"sr_{b}_{h}_{n0}",
                                             tag="sm_r")
                                nc.vector.scalar_tensor_tensor(out=r[:nw, :], in0=qf[:nw, :],
                                                               scalar=negx0_b[:nw, 0:1],
                                                               in1=x[:nw, :], op0=op.mult,
                                                               op1=op.add)
                                t = smp.tile([128, N], dt.float32, name=f"stp_{b}_{h}_{n0}",
                                             tag="sm_t")
                                nc.vector.scalar_tensor_tensor(out=t[:nw, :], in0=r[:nw, :],
                                                               scalar=bi_b[:nw, 0:1],
                                                               in1=r[:nw, :], op0=op.add,
                                                               op1=op.mult)
                                ei = smp.tile([128, N], dt.int32, name=f"sei_{b}_{h}_{n0}",
                                              tag="sm_ei")
                                nc.vector.tensor_scalar(out=ei[:nw, :], in0=qi32[:nw, :],
                                                        scalar1=-1, scalar2=157, op0=op.mult,
                                                        op1=op.add)
                                nc.vector.tensor_scalar(out=ei[:nw, :], in0=ei[:nw, :],
                                                        scalar1=23, scalar2=None,
                                                        op0=op.logical_shift_left)
                                ex = smp.tile([128, N], dt.float32, name=f"sex_{b}_{h}_{n0}",
                                              tag="sm_ex")
                                nc.vector.scalar_tensor_tensor(
                                    out=ex[:nw, :], in0=t[:nw, :], scalar=ci_b[:nw, 0:1],
                                    in1=ei[:nw, :].bitcast(dt.float32), op0=op.add, op1=op.mult)
                                rsum = smp.tile([128, 1], dt.float32, name=f"ssu_{b}_{h}_{n0}",
                                                tag="sm_su")
                                nc.vector.tensor_scalar(out=ex[:nw, :], in0=ex[:nw, :],
                                                        scalar1=0.0, scalar2=None, op0=op.max,
                                                        op1=op.add, accum_out=rsum[:nw, 0:1])
                                rp = smp.tile([128, 1], dt.float32, name=f"srp_{b}_{h}_{n0}",
                                              tag="sm_rp")
                                nc.vector.reciprocal(out=rp[:nw], in_=rsum[:nw])
                                nc.vector.tensor_scalar(out=rp[:nw], in0=rp[:nw],
                                                        scalar1=float(2.0 ** 32), scalar2=0.49,
                                                        op0=op.mult, op1=op.subtract)
                                rpi = smp.tile([128, 1], dt.int32, name=f"srpi_{b}_{h}_{n0}",
                                               tag="sm_rpi")
                                nc.vector.tensor_copy(out=rpi[:nw], in_=rp[:nw])
                                nc.vector.tensor_copy(out=rp[:nw], in_=rpi[:nw])
                                nc.vector.tensor_scalar(out=rp[:nw], in0=rp[:nw],
                                                        scalar1=float(2.0 ** -16), scalar2=None,
                                                        op0=op.mult)
                                nc.vector.tensor_scalar(out=ex[:nw, :], in0=ex[:nw, :],
                                                        scalar1=rp[:nw, 0:1], scalar2=0.49,
                                                        op0=op.mult, op1=op.subtract)
                                exi = smp.tile([128, N], dt.int32, name=f"sxe_{b}_{h}_{n0}",
                                               tag="sm_xe")
                                nc.vector.tensor_copy(out=exi[:nw, :], in_=ex[:nw, :])
                                nc.vector.tensor_copy(out=ex[:nw, :], in_=exi[:nw, :])
                                for (m0, mw) in NS:
                                    pa = pt2.tile([128, 128], dt.float32,
                                                  name=f"pa_{b}_{h}_{n0}_{m0}", tag="pa")
                                    nc.tensor.transpose(pa[:mw, :nw], ex[:nw, m0:m0 + mw],
                                                        idf[:nw, :nw])
                                    at = smp.tile([128, 128], dt.float32,
                                                  name=f"at_{b}_{h}_{n0}_{m0}",
                                                  tag=f"at_{m0}_{n0}")
                                    nc.scalar.activation(out=at[:mw, :nw], in_=pa[:mw, :nw],
                                                         func=act.Copy)
                                    at_parts[(m0, n0)] = at
                            pav = pt2.tile([64, N], dt.float32, name=f"pav_{b}_{h}", tag="pav")
                            for mi, (m0, mw) in enumerate(NS):
                                rhs = smp.tile([128, N], dt.float32, name=f"rhs_{b}_{h}_{m0}",
                                               tag=f"rhs_{m0}")
                                for (n0, nw) in NS:
                                    nc.vector.tensor_copy(out=rhs[:mw, n0:n0 + nw],
                                                          in_=at_parts[(m0, n0)][:mw, :nw])
                                nc.tensor.matmul(pav[:, :], vtok[mi][:mw, :], rhs[:mw, :],
                                                 start=(mi == 0), stop=(mi == 1))
                            ov = smp.tile([64, N], dt.float32, name=f"ov_{b}_{h}", tag="ov")
                            nc.scalar.activation(out=ov[:, :], in_=pav[:, :], func=act.Copy)
                            nc.sync.dma_start(
                                out=av_spill[h * 64:(h + 1) * 64, b * N:(b + 1) * N], in_=ov)
                            red = smp.tile([64, 1], dt.float32, name=f"rav_{b}_{h}", tag="rav")
                            nc.vector.tensor_reduce(out=red[:64], in_=pav[:, :], axis=AX.X,
                                                    op=op.max, apply_absolute_value=True)
                            mx_av.add(red[:64])
            av_lmax = mx_av.finish()
            g4 = allreduce_max(av_lmax, "qa4")   # 0 in the realistic case
            sf_av_in = sc_op(sf1a_g, op.mult, float(2.0 ** -16))
            sf_av = sc_op(sc_mul2(g4, sf_av_in), op.mult, float(_f32(1.0 / 127.0)))
            sf_av_g = sc_op(sf_av, op.max, 1e-37)
            rcp_av = sc_recip(sf_av_g)
            ratio_av_b = bcast(sc_mul2(sf_av_in, rcp_av), "ravb")

            with tc.tile_pool(name="aqp", bufs=1) as aqp:
                attn_q = [aqp.tile([128, M], dt.bfloat16, name=f"aq_{c}", tag=f"aq_{c}")
                          for c in range(CT)]
                for c in range(CT):
                    for (c0, cw) in CHUNKS:
                        sb = tp.tile([128, CHUNK], dt.float32, name=f"aqs_{c}_{c0}", tag="mm_u")
                        nc.sync.dma_start(out=sb[:, :cw],
                                          in_=av_spill[c * PT:(c + 1) * PT, c0:c0 + cw])
                        qi = tp.tile([128, CHUNK], dt.int32, name=f"aqi_{c}_{c0}", tag="mm_ui")
                        nc.scalar.activation(out=qi[:, :cw], in_=sb[:, :cw], func=act.Copy,
                                             scale=ratio_av_b[:, 0:1])
                        nc.vector.tensor_copy(out=attn_q[c][:, c0:c0 + cw], in_=qi[:, :cw])

                # ---- proj ----
                bip = floor_cm(bpw_cm, CT, bcast(rcp_av, "rav2"), "bip")
                pj_spill = dr.tile([C, M], dt.float32, name="pj_spill")
                mx_pj = MaxAcc("proj")
                for otc in range(CT):
                    wts = []
                    for kt in range(CT):
                        w = wp.tile([128, 128], dt.bfloat16, name=f"wpj_{otc}_{kt}", tag="wq")
                        nc.sync.dma_start(out=w, in_=wprojT_d[kt * PT:(kt + 1) * PT,
                                                             otc * PT:(otc + 1) * PT])
                        wts.append(w)
                    for (c0, cw) in CHUNKS:
                        ps = pp.tile([128, CHUNK], dt.float32, name=f"ppj_{otc}_{c0}", tag="pq")
                        for kt in range(CT):
                            nc.tensor.matmul(ps[:, :cw], wts[kt][:, :],
                                             attn_q[kt][:, c0:c0 + cw],
                                             start=(kt == 0), stop=(kt == CT - 1))
                        sb = tp.tile([128, CHUNK], dt.float32, name=f"spj_{otc}_{c0}",
                                     tag="mm_sb")
                        nc.vector.tensor_scalar(out=sb[:, :cw], in0=ps[:, :cw],
                                                scalar1=bip[:, otc:otc + 1], scalar2=None,
                                                op0=op.add)
                        red = tp.tile([128, 1], dt.float32, name=f"rpj_{otc}_{c0}", tag="mm_red")
                        nc.vector.tensor_reduce(out=red, in_=sb[:, :cw], axis=AX.X, op=op.max,
                                                apply_absolute_value=True)
                        nc.vector.tensor_tensor(out=red, in0=red, in1=wsf_cm[:, otc:otc + 1],
                                                op=op.mult)
                        mx_pj.add(red)
                        nc.sync.dma_start(out=pj_spill[otc * PT:(otc + 1) * PT, c0:c0 + cw],
                                          in_=sb[:, :cw])
                pj_lmax = mx_pj.finish()
            g5 = allreduce_max(pj_lmax, "qa5")
            sf5 = sc_op(sc_mul2(g5, sf_av_g), op.mult, float(_f32(1.0 / 32767.0)))
            sf5_g = sc_op(sf5, op.max, 1e-37)
            t_r5 = sing.tile([128, CT], dt.float32, name="t_r5")
            nc.vector.tensor_scalar(out=t_r5, in0=wsf_cm,
                                    scalar1=bcast(sf_av_g, "sav3")[:, 0:1],
                                    scalar2=bcast(sc_recip(sf5_g), "r5b")[:, 0:1],
                                    op0=op.mult, op1=op.mult)

            # quantize proj, transpose to token-major, resid1
            r1_spill = dr.tile([M, C], dt.float32, name="r1_spill")
            sf5_b = bcast(sf5_g, "sf5b")
            mx_r1 = MaxAcc("r1")
            for it in range(NT):
                p = min(PT, M - it * PT)
                ytok = tp.tile([128, C], dt.float32, name=f"ytk_{it}", tag="ytk")
                for c in range(CT):
                    sb = tp.tile([128, 128], dt.float32, name=f"pjl_{it}_{c}", tag="pjl")
                    nc.sync.dma_start(out=sb[:, :p], in_=pj_spill[c * PT:(c + 1) * PT,
                                                                  it * PT:it * PT + p])
                    qi = tp.tile([128, 128], dt.int32, name=f"pji_{it}_{c}", tag="pji")
                    nc.scalar.activation(out=qi[:, :p], in_=sb[:, :p], func=act.Copy,
                                         scale=t_r5[:, c:c + 1])
                    qf = tp.tile([128, 128], dt.float32, name=f"pjf_{it}_{c}", tag="pjf")
                    nc.vector.tensor_copy(out=qf[:, :p], in_=qi[:, :p])
                    if p < 128:
                        nc.vector.memset(qf[:, p:], 0.0)
                    pa = pt2.tile([128, 128], dt.float32, name=f"pjt_{it}_{c}", tag="pa")
                    nc.tensor.transpose(pa[:, :], qf[:, :], idf[:])
                    nc.scalar.activation(out=ytok[:p, c * PT:(c + 1) * PT], in_=pa[:p, :],
                                         func=act.Copy)
                x1t = tp.tile([128, C], dt.float32, name=f"x1r_{it}", tag="ln_xin")
                nc.sync.dma_start(out=x1t[:p, :], in_=x1_d[it * PT:it * PT + p, :])
                nc.vector.scalar_tensor_tensor(out=ytok[:p, :], in0=ytok[:p, :],
                                               scalar=sf5_b[:p, 0:1], in1=x1t[:p, :],
                                               op0=op.mult, op1=op.add)
                red = tp.tile([128, 1], dt.float32, name=f"rr1_{it}", tag="mm_red")
                mx_r1.add_from(ytok[:p, :], red[:p])
                nc.sync.dma_start(out=r1_spill[it * PT:it * PT + p, :], in_=ytok[:p, :])
            r1_lmax = mx_r1.finish()
            g6 = allreduce_max(r1_lmax, "qa6")
            sf2 = sc_op(g6, op.mult, float(_f32(1.0 / 32767.0)))
            sf2_g = sc_op(sf2, op.max, 1e-37)
            rcp2_b = bcast(sc_recip(sf2_g), "rcp2")
            x2q_spill = dr.tile([M, C], dt.float32, name="x2q_spill")
            for it in range(NT):
                p = min(PT, M - it * PT)
                yb = tp.tile([128, C], dt.float32, name=f"x2l_{it}", tag="qt_y")
                nc.sync.dma_start(out=yb[:p, :], in_=r1_spill[it * PT:it * PT + p, :])
                qi = tp.tile([128, C], dt.int32, name=f"x2i_{it}", tag="qt_qi")
                nc.scalar.activation(out=qi[:p, :], in_=yb[:p, :], func=act.Copy,
                                     scale=rcp2_b[:p, 0:1])
                qf = tp.tile([128, C], dt.float32, name=f"x2f_{it}", tag="x2f")
                nc.vector.tensor_copy(out=qf[:p, :], in_=qi[:p, :])
                nc.sync.dma_start(out=x2q_spill[it * PT:it * PT + p, :], in_=qf[:p, :])

            # ========================= LN2 + qa7 =========================
            ln2_spill, ln2_lmax = ln_stage(x2q_spill, ln2bi, ln2sc, None, "l2")
            g7 = allreduce_max(ln2_lmax, "qa7")
            sf7 = sc_op(g7, op.mult, float(_f32(1.0 / 127.0)))
            sf7_g = sc_op(sf7, op.max, 1e-37)
            rcp7_b = bcast(sc_recip(sf7_g), "rcp7")

            out_sf_fc1 = sc_mul2(sf7, scals[0:1, 2:3])
            out_sf_fc1_g = sc_op(out_sf_fc1, op.max, 1e-37)

            with tc.tile_pool(name="mats2", bufs=1) as mats2:
                xqT2 = [mats2.tile([128, M], dt.bfloat16, name=f"xqT2_{c}", tag=f"xqT2_{c}")
                        for c in range(CT)]
                quant_transpose(ln2_spill, rcp7_b, xqT2, "x2")

                # ===================== FC1 =====================
                bf1 = floor_cm(bfc1_cm, OT_FC1, bcast(sc_recip(out_sf_fc1_g), "rosf1"), "bf1")
                fc1_spill = dr.tile([HID, M], dt.float32, name="fc1_spill")
                mx_f1 = MaxAcc("fc1")
                for ot in range(OT_FC1):
                    wts = []
                    for kt in range(CT):
                        w = wp.tile([128, 128], dt.bfloat16, name=f"wf1_{ot}_{kt}", tag="wq")
                        nc.sync.dma_start(out=w, in_=wfc1T_d[kt * PT:(kt + 1) * PT,
                                                            ot * PT:(ot + 1) * PT])
                        wts.append(w)
                    for (c0, cw) in CHUNKS:
                        ps = pp.tile([128, CHUNK], dt.float32, name=f"pf1_{ot}_{c0}", tag="pq")
                        for kt in range(CT):
                            nc.tensor.matmul(ps[:, :cw], wts[kt][:, :], xqT2[kt][:, c0:c0 + cw],
                                             start=(kt == 0), stop=(kt == CT - 1))
                        sb = tp.tile([128, CHUNK], dt.float32, name=f"sf1_{ot}_{c0}",
                                     tag="mm_sb")
                        nc.vector.tensor_scalar(out=sb[:, :cw], in0=ps[:, :cw],
                                                scalar1=bf1[:, ot:ot + 1], scalar2=None,
                                                op0=op.add)
                        red = tp.tile([128, 1], dt.float32, name=f"rf1_{ot}_{c0}", tag="mm_red")
                        mx_f1.add_from(sb[:, :cw], red)
                        nc.sync.dma_start(out=fc1_spill[ot * PT:(ot + 1) * PT, c0:c0 + cw],
                                          in_=sb[:, :cw])
                f1_lmax = mx_f1.finish()
            g8 = allreduce_max(f1_lmax, "qa8")
            sf8 = sc_op(sc_mul2(g8, out_sf_fc1_g), op.mult, float(_f32(1.0 / 127.0)))
            sf8_g = sc_op(sf8, op.max, 1e-37)
            ratio_f1_b = bcast(sc_mul2(out_sf_fc1_g, sc_recip(sf8_g)), "rf1b")

            rcp_e = sc_op(sc_recip(sf8_g), op.mult, 1.4142)
            bi_g = sc_floor(sc_op(rcp_e, op.mult, -1.769))
            nbi_g_b = bcast(sc_op(bi_g, op.mult, -1.0), "nbig")
            bi_g_b = bcast(bi_g, "big")
            ci_gf = sc_op(sc_mul2(rcp_e, rcp_e), op.mult, float(_f32(1.0 / -0.2888)))
            ci_g = sc_floor(ci_gf)
            ci_g_b = bcast(ci_g, "cig")
            shift_b = ci_g_b  # floor(1/erf_sf) == floor(ci_gf) == ci_g

            gelu_spill = dr.tile([HID, M], dt.float32, name="gelu_spill")
            mx_ge = MaxAcc("gelu")
            with tc.tile_pool(name="gep", bufs=2) as gep:
                for ot in range(OT_FC1):
                    for (c0, cw) in CHUNKS:
                        sb = tp.tile([128, CHUNK], dt.float32, name=f"gi_{ot}_{c0}", tag="mm_u")
                        nc.sync.dma_start(out=sb[:, :cw],
                                          in_=fc1_spill[ot * PT:(ot + 1) * PT, c0:c0 + cw])
                        qi = tp.tile([128, CHUNK], dt.int32, name=f"gqi_{ot}_{c0}", tag="mm_ui")
                        nc.scalar.activation(out=qi[:, :cw], in_=sb[:, :cw], func=act.Copy,
                                             scale=ratio_f1_b[:, 0:1])
                        xg = gep.tile([128, CHUNK], dt.float32, name=f"gx_{ot}_{c0}", tag="gx")
                        nc.vector.tensor_copy(out=xg[:, :cw], in_=qi[:, :cw])
                        sg = gep.tile([128, CHUNK], dt.float32, name=f"gs_{ot}_{c0}", tag="gs")
                        nc.scalar.activation(out=sg[:, :cw], in_=xg[:, :cw], func=act.Sign)
                        ab = gep.tile([128, CHUNK], dt.float32, name=f"ga_{ot}_{c0}", tag="ga")
                        nc.scalar.activation(out=ab[:, :cw], in_=xg[:, :cw], func=act.Abs)
                        nc.vector.tensor_scalar(out=ab[:, :cw], in0=ab[:, :cw],
                                                scalar1=nbi_g_b[:, 0:1],
                                                scalar2=bi_g_b[:, 0:1],
                                                op0=op.min, op1=op.add)
                        sq = gep.tile([128, CHUNK], dt.float32, name=f"gq2_{ot}_{c0}", tag="gq2")
                        nc.scalar.activation(out=sq[:, :cw], in_=ab[:, :cw], func=act.Square)
                        nc.vector.scalar_tensor_tensor(out=sq[:, :cw], in0=sq[:, :cw],
                                                       scalar=ci_g_b[:, 0:1], in1=sg[:, :cw],
                                                       op0=op.add, op1=op.mult)
                        nc.vector.scalar_tensor_tensor(out=xg[:, :cw], in0=sq[:, :cw],
                                                       scalar=shift_b[:, 0:1], in1=xg[:, :cw],
                                                       op0=op.add, op1=op.mult)
                        red = tp.tile([128, 1], dt.float32, name=f"rge_{ot}_{c0}", tag="mm_red")
                        mx_ge.add_from(xg[:, :cw], red)
                        nc.sync.dma_start(out=gelu_spill[ot * PT:(ot + 1) * PT, c0:c0 + cw],
                                          in_=xg[:, :cw])
            ge_lmax = mx_ge.finish()
            g9 = allreduce_max(ge_lmax, "qa9")
            sf_ge_out = sc_op(sc_mul2(sf8_g, sc_recip(ci_gf)), op.mult, 0.5)  # negative
            sf9 = sc_op(sc_mul2(g9, sc_op(sf_ge_out, op.mult, -1.0)), op.mult,
                        float(_f32(1.0 / 127.0)))
            sf9_g = sc_op(sf9, op.max, 1e-37)
            ratio_ge_b = bcast(sc_mul2(sf_ge_out, sc_recip(sf9_g)), "rgeb")

            with tc.tile_pool(name="geq", bufs=1) as geqp:
                xq_ge = [geqp.tile([128, M], dt.bfloat16, name=f"xge_{t}", tag=f"xge_{t}")
                         for t in range(OT_FC1)]
                for ot in range(OT_FC1):
                    for (c0, cw) in CHUNKS:
                        sb = tp.tile([128, CHUNK], dt.float32, name=f"ge2_{ot}_{c0}", tag="mm_u")
                        nc.sync.dma_start(out=sb[:, :cw],
                                          in_=gelu_spill[ot * PT:(ot + 1) * PT, c0:c0 + cw])
                        qi = tp.tile([128, CHUNK], dt.int32, name=f"ge2i_{ot}_{c0}",
                                     tag="mm_ui")
                        nc.scalar.activation(out=qi[:, :cw], in_=sb[:, :cw], func=act.Copy,
                                             scale=ratio_ge_b[:, 0:1])
                        nc.vector.tensor_copy(out=xq_ge[ot][:, c0:c0 + cw], in_=qi[:, :cw])

                # ===================== FC2 =====================
                out_sf_fc2 = sc_mul2(sf9, scals[0:1, 3:4])
                out_sf_fc2_g = sc_op(out_sf_fc2, op.max, 1e-37)
                bf2 = floor_cm(bfc2_cm, CT, bcast(sc_recip(out_sf_fc2_g), "rosf2"), "bf2")
                f2_spill = dr.tile([C, M], dt.float32, name="f2_spill")
                mx_f2 = MaxAcc("fc2")
                for otc in range(CT):
                    wts = []
                    for kt in range(OT_FC1):
                        w = wp2.tile([128, 128], dt.bfloat16, name=f"wf2_{otc}_{kt}", tag="wq2")
                        nc.sync.dma_start(out=w, in_=wfc2T_d[kt * PT:(kt + 1) * PT,
                                                            otc * PT:(otc + 1) * PT])
                        wts.append(w)
                    for (c0, cw) in CHUNKS:
                        ps = pp.tile([128, CHUNK], dt.float32, name=f"pf2_{otc}_{c0}", tag="pq")
                        for kt in range(OT_FC1):
                            nc.tensor.matmul(ps[:, :cw], wts[kt][:, :],
                                             xq_ge[kt][:, c0:c0 + cw],
                                             start=(kt == 0), stop=(kt == OT_FC1 - 1))
                        sb = tp.tile([128, CHUNK], dt.float32, name=f"sf2_{otc}_{c0}",
                                     tag="mm_sb")
                        nc.vector.tensor_scalar(out=sb[:, :cw], in0=ps[:, :cw],
                                                scalar1=bf2[:, otc:otc + 1], scalar2=None,
                                                op0=op.add)
                        red = tp.tile([128, 1], dt.float32, name=f"rf2_{otc}_{c0}",
                                      tag="mm_red")
                        mx_f2.add_from(sb[:, :cw], red)
                        nc.sync.dma_start(out=f2_spill[otc * PT:(otc + 1) * PT, c0:c0 + cw],
                                          in_=sb[:, :cw])
                f2_lmax = mx_f2.finish()
            g10 = allreduce_max(f2_lmax, "qa10")
            sf10 = sc_op(sc_mul2(g10, out_sf_fc2_g), op.mult, float(_f32(1.0 / 32767.0)))
            sf10_g = sc_op(sf10, op.max, 1e-37)
            ratio_f2_b = bcast(sc_mul2(out_sf_fc2_g, sc_recip(sf10_g)), "rf2b")

            r2_spill = dr.tile([M, C], dt.float32, name="r2_spill")
            sf10_b = bcast(sf10_g, "s10b")
            sf2_b2 = bcast(sf2_g, "s2b2")
            mx_r2 = MaxAcc("r2")
            for it in range(NT):
                p = min(PT, M - it * PT)
                ytok = tp.tile([128, C], dt.float32, name=f"y2tk_{it}", tag="ytk")
                for c in range(CT):
                    sb = tp.tile([128, 128], dt.float32, name=f"f2l_{it}_{c}", tag="pjl")
                    nc.sync.dma_start(out=sb[:, :p], in_=f2_spill[c * PT:(c + 1) * PT,
                                                                  it * PT:it * PT + p])
                    qi = tp.tile([128, 128], dt.int32, name=f"f2i_{it}_{c}", tag="pji")
                    nc.scalar.activation(out=qi[:, :p], in_=sb[:, :p], func=act.Copy,
                                         scale=ratio_f2_b[:, 0:1])
                    qf = tp.tile([128, 128], dt.float32, name=f"f2f_{it}_{c}", tag="pjf")
                    nc.vector.tensor_copy(out=qf[:, :p], in_=qi[:, :p])
                    if p < 128:
                        nc.vector.memset(qf[:, p:], 0.0)
                    pa = pt2.tile([128, 128], dt.float32, name=f"f2t_{it}_{c}", tag="pa")
                    nc.tensor.transpose(pa[:, :], qf[:, :], idf[:])
                    nc.scalar.activation(out=ytok[:p, c * PT:(c + 1) * PT], in_=pa[:p, :],
                                         func=act.Copy)
                x2t = tp.tile([128, C], dt.float32, name=f"x2r_{it}", tag="ln_xin")
                nc.sync.dma_start(out=x2t[:p, :], in_=x2q_spill[it * PT:it * PT + p, :])
                nc.vector.tensor_scalar(out=x2t[:p, :], in0=x2t[:p, :],
                                        scalar1=sf2_b2[:p, 0:1], scalar2=None, op0=op.mult)
                nc.vector.scalar_tensor_tensor(out=ytok[:p, :], in0=ytok[:p, :],
                                               scalar=sf10_b[:p, 0:1], in1=x2t[:p, :],
                                               op0=op.mult, op1=op.add)
                red = tp.tile([128, 1], dt.float32, name=f"rr2_{it}", tag="mm_red")
                mx_r2.add_from(ytok[:p, :], red[:p])
                nc.sync.dma_start(out=r2_spill[it * PT:it * PT + p, :], in_=ytok[:p, :])
            r2_lmax = mx_r2.finish()
            g11 = allreduce_max(r2_lmax, "qa11")
            sff = sc_op(g11, op.mult, float(_f32(1.0 / 32767.0)))
            sff_g = sc_op(sff, op.max, 1e-37)
            rcpf_b = bcast(sc_recip(sff_g), "rcpf")
            sff_b = bcast(sff_g, "sffb")
            for it in range(NT):
                p = min(PT, M - it * PT)
                yb = tp.tile([128, C], dt.float32, name=f"fol_{it}", tag="qt_y")
                nc.sync.dma_start(out=yb[:p, :], in_=r2_spill[it * PT:it * PT + p, :])
                qi = tp.tile([128, C], dt.int32, name=f"fo_{it}", tag="qt_qi")
                nc.scalar.activation(out=qi[:p, :], in_=yb[:p, :], func=act.Copy,
                                     scale=rcpf_b[:p, 0:1])
                fo = tp.tile([128, C], dt.float32, name=f"fof_{it}", tag="x2f")
                nc.vector.tensor_copy(out=fo[:p, :], in_=qi[:p, :])
                nc.vector.tensor_scalar(out=fo[:p, :], in0=fo[:p, :], scalar1=sff_b[:p, 0:1],
                                        scalar2=None, op0=op.mult)
                nc.sync.dma_start(out=out_d[it * PT:it * PT + p, :], in_=fo[:p, :])

            st = sing.tile([1, 16], dt.float32, name="st")
            nc.vector.memset(st, 0.0)
            for i, s in enumerate([g4, sff, sf_qa1, sf1a, sf_s, sf2, sf5, sf7, sf9, sf10]):
                nc.vector.tensor_copy(out=st[0:1, i:i + 1], in_=s[0:1, 0:1])
            nc.sync.dma_start(out=stats_d[:], in_=st)

    nc.finalize()
    return nc


def _get_module():
    global _BUILT
    if _BUILT is None:
        _BUILT = _build_module()
    return _BUILT


_TRACE = False
_LAST_RESULT = [None]


def kernel(x_1, act_scaling_factor_1, ln1_w, ln1_b, w_qkv, w_proj, b_proj,
           ln2_w, ln2_b, w_fc1, b_fc1, w_fc2, b_fc2):
    x_1 = np.asarray(x_1, np.float32)
    sf1 = _f32(np.asarray(act_scaling_factor_1).reshape(()))

    wq_ter, alpha_q = _ternarize(w_qkv)
    wf1_ter, alpha_1 = _ternarize(w_fc1)
    wf2_ter, alpha_2 = _ternarize(w_fc2)
    wp_int, wp_sf = _quant_w(w_proj)

    to_bf = lambda a: np.ascontiguousarray(a.T).astype(ml_dtypes.bfloat16)
    wqkvT = to_bf(wq_ter)
    wprojT = to_bf(wp_int)
    wfc1T = to_bf(wf1_ter)
    wfc2T = to_bf(wf2_ter)

    sfp = _f32(math.sqrt(C)) / _f32(2.0 ** 30)
    ln1bi = np.floor((np.float32(ln1_b) / np.float32(ln1_w)) / sfp).astype(np.float32)[None, :]
    ln1sc = (sfp * np.float32(ln1_w)).astype(np.float32)[None, :]
    ln2bi = np.floor((np.float32(ln2_b) / np.float32(ln2_w)) / sfp).astype(np.float32)[None, :]
    ln2sc = (sfp * np.float32(ln2_w)).astype(np.float32)[None, :]
    bpw = (np.float32(b_proj) / np.maximum(wp_sf, 1e-37)).astype(np.float32)[None, :]

    scal = np.zeros((1, 8), np.float32)
    scal[0, 0] = _f32(1.0) / sf1
    scal[0, 1] = alpha_q
    scal[0, 2] = alpha_1
    scal[0, 3] = alpha_2

    nc = _get_module()
    from concourse.bass_utils import run_bass_kernel_spmd

    shared = dict(
        wqkvT=wqkvT, wprojT=wprojT, wfc1T=wfc1T, wfc2T=wfc2T,
        ln1bi=ln1bi, ln1sc=ln1sc, ln2bi=ln2bi, ln2sc=ln2sc,
        bpw=bpw, wsf=wp_sf.astype(np.float32)[None, :],
        bfc1=np.float32(b_fc1)[None, :], bfc2=np.float32(b_fc2)[None, :],
        scal=scal,
    )
    in_maps = []
    for c in range(NCORES):
        m = dict(shared)
        m["x1"] = np.ascontiguousarray(x_1[c * PER:(c + 1) * PER].reshape(M, C))
        in_maps.append(m)

    kw = {"trace": True} if _TRACE else {}
    res = run_bass_kernel_spmd(nc, in_maps, core_ids=list(range(NCORES)), **kw)
    _LAST_RESULT[0] = res

    stats = res.results[0]["stats"][0]
    max_av_int = float(stats[0])

    # Reference semantics: quant_act after attn@v computes sf = max|x|/127. If
    # that max is 0 (softmax collapsed -- the realistic case), x/sf = 0/0 = NaN
    # poisons every element downstream: output = all-NaN, sf = NaN.
    if max_av_int == 0.0 or not np.isfinite(max_av_int):
        return np.full((B, N, C), np.nan, dtype=np.float32), np.float32(np.nan)

    out = np.empty((B, N, C), np.float32)
    for c in range(NCORES):
        out[c * PER:(c + 1) * PER] = res.results[c]["out"].reshape(PER, N, C)
    return out, np.float32(stats[1])


# revision 9
# speedup vs baseline: 1.0157x; 1.0157x over previous
"""Trainium2 Bass kernel for the quantized (I-BERT style) ViT block.

kernel(**inputs) takes the FULL unsharded inputs (as in setup_inputs()) and
returns the FULL output matching reference(**inputs) -> (x, scaling_factor).

Data-parallel over batch across 8 NeuronCores (8 images / core). Per-tensor
activation scale factors use a scalar AllReduce(max) at each quantization
point. Stage outputs that must wait for a global max are spilled to DRAM and
re-streamed for the quantize pass (SBUF cannot hold them all).

Degenerate path note: the reference's integer softmax collapses for any
realistic data (exp_sum >> 2^32 so factor=0), making attn@v exactly 0; the
reference's quant_act then computes 0/0 = NaN and the output is all-NaN.  The
device computes the same pipeline with guarded reciprocals (hardware clamps
instead of producing IEEE NaN); the host applies the reference's exact
semantics when the device-computed global max is 0.
"""

import math

import numpy as np
import ml_dtypes

B, N, C, HID, H = 64, 197, 768, 3072, 12
D = C // H
NCORES = 8
PER = B // NCORES
M = PER * N                # 1576 tokens per core
PT = 128
NT = (M + PT - 1) // PT    # 13
CT = C // PT               # 6
OT_QKV = 3 * C // PT       # 18
OT_FC1 = HID // PT         # 24
CHUNK = 512
CHUNKS = [(i, min(CHUNK, M - i)) for i in range(0, M, CHUNK)]


def _f32(x):
    return np.float32(x)


def _ternarize(w):
    w = np.asarray(w, np.float32)
    thr = _f32(0.7) * _f32(np.mean(np.abs(w)))
    mask = (np.abs(w) > thr).astype(np.float32)
    alpha = _f32(np.sum(np.abs(w) * mask) / max(np.sum(mask), 1.0))
    return np.sign(w).astype(np.float32) * mask, alpha


def _quant_w(w):
    w = np.asarray(w, np.float32)
    w_sf = np.max(np.abs(w), axis=1).astype(np.float32) / _f32(127.0)
    w_int = np.float32(np.round(w / w_sf[:, None]))
    return w_int, w_sf


_BUILT = None


def _build_module():
    import concourse.bass as bass
    import concourse.mybir as mybir
    import concourse.tile as tile
    from concourse import bacc
    from concourse.masks import make_identity

    dt = mybir.dt
    op = mybir.AluOpType
    act = mybir.ActivationFunctionType
    AX = mybir.AxisListType

    nc = bacc.Bacc("TRN2", target_bir_lowering=False, debug=False)

    x1_d = nc.dram_tensor("x1", [M, C], dt.float32, kind="ExternalInput")
    wqkvT_d = nc.dram_tensor("wqkvT", [C, 3 * C], dt.bfloat16, kind="ExternalInput")
    wprojT_d = nc.dram_tensor("wprojT", [C, C], dt.bfloat16, kind="ExternalInput")
    wfc1T_d = nc.dram_tensor("wfc1T", [C, HID], dt.bfloat16, kind="ExternalInput")
    wfc2T_d = nc.dram_tensor("wfc2T", [HID, C], dt.bfloat16, kind="ExternalInput")
    ln1bi_d = nc.dram_tensor("ln1bi", [1, C], dt.float32, kind="ExternalInput")
    ln1sc_d = nc.dram_tensor("ln1sc", [1, C], dt.float32, kind="ExternalInput")
    ln2bi_d = nc.dram_tensor("ln2bi", [1, C], dt.float32, kind="ExternalInput")
    ln2sc_d = nc.dram_tensor("ln2sc", [1, C], dt.float32, kind="ExternalInput")
    bpw_d = nc.dram_tensor("bpw", [1, C], dt.float32, kind="ExternalInput")
    wsf_d = nc.dram_tensor("wsf", [1, C], dt.float32, kind="ExternalInput")
    bfc1_d = nc.dram_tensor("bfc1", [1, HID], dt.float32, kind="ExternalInput")
    bfc2_d = nc.dram_tensor("bfc2", [1, C], dt.float32, kind="ExternalInput")
    scal_d = nc.dram_tensor("scal", [1, 8], dt.float32, kind="ExternalInput")
    out_d = nc.dram_tensor("out", [M, C], dt.float32, kind="ExternalOutput")
    stats_d = nc.dram_tensor("stats", [1, 16], dt.float32, kind="ExternalOutput")

    with tile.TileContext(nc) as tc:
        import contextlib
        ctx = contextlib.ExitStack()
        with ctx:
            sing = ctx.enter_context(tc.tile_pool(name="sing", bufs=1))
            scp = ctx.enter_context(tc.tile_pool(name="scp", bufs=1))
            dr = ctx.enter_context(tc.tile_pool(name="dr", bufs=1, space="DRAM"))
            tp = ctx.enter_context(tc.tile_pool(name="tp", bufs=2))
            wp = ctx.enter_context(tc.tile_pool(name="wp", bufs=8))
            wp2 = ctx.enter_context(tc.tile_pool(name="wp2", bufs=26))
            pp = ctx.enter_context(tc.tile_pool(name="pp", bufs=2, space="PSUM"))
            pt2 = ctx.enter_context(tc.tile_pool(name="pt2", bufs=1, space="PSUM"))
            pss = ctx.enter_context(tc.tile_pool(name="pss", bufs=2, space="PSUM"))

            idf = sing.tile([128, 128], dt.float32, name="idf")
            make_identity(nc, idf)
            idb = sing.tile([128, 128], dt.bfloat16, name="idb")
            make_identity(nc, idb)

            def bcast_dram(dten, name, offset=0):
                b = scp.tile([128, 1], dt.float32, name=f"b_{name}", tag=f"b_{name}")
                nc.sync.dma_start(
                    out=b, in_=bass.AP(tensor=dten, offset=offset, ap=[[0, 128], [1, 1]])
                )
                return b

            def bcast(src_ap, name):
                d = dr.tile([1, 1], dt.float32, name=f"d_{name}", tag=f"d_{name}")
                nc.sync.dma_start(out=d, in_=src_ap)
                return bcast_dram(d.tensor, name, d.offset)

            cc_n = [0]

            def allreduce_max(local_ap, name):
                cc_n[0] += 1
                i = cc_n[0]
                di = dr.tile([1, 1], dt.float32, name=f"cci_{i}", tag=f"cci_{i}")
                do = dr.tile([1, 1], dt.float32, name=f"cco_{i}", tag=f"cco_{i}",
                             addr_space="Shared")
                nc.sync.dma_start(out=di, in_=local_ap)
                nc.gpsimd.collective_compute(
                    "AllReduce", op.max, replica_groups=[list(range(NCORES))],
                    ins=[di.opt()], outs=[do.opt()],
                )
                g = scp.tile([1, 1], dt.float32, name=f"ccg_{i}", tag=f"ccg_{i}")
                nc.sync.dma_start(out=g, in_=do)
                return g

            sc_i = [0]

            def sc_tile(n=1):
                sc_i[0] += 1
                return scp.tile([1, n], dt.float32, name=f"s{sc_i[0]}", tag=f"s{sc_i[0]}")

            def sc_op(a_ap, alu, s1, s2=None, alu2=None):
                o = sc_tile()
                nc.vector.tensor_scalar(out=o, in0=a_ap, scalar1=s1, scalar2=s2, op0=alu,
                                        **({"op1": alu2} if alu2 is not None else {}))
                return o

            def sc_recip(a_ap):
                o = sc_tile()
                nc.vector.reciprocal(out=o, in_=a_ap)
                return o

            def sc_mul2(a_ap, b_ap):
                o = sc_tile()
                nc.vector.tensor_tensor(out=o, in0=a_ap, in1=b_ap, op=op.mult)
                return o

            def sc_floor(a_ap):
                sc_i[0] += 1
                i = scp.tile([1, 1], dt.int32, name=f"fi{sc_i[0]}", tag=f"fi{sc_i[0]}")
                nc.vector.tensor_copy(out=i, in_=a_ap)
                f = sc_tile()
                nc.vector.tensor_copy(out=f, in_=i)
                g = sc_tile()
                nc.vector.tensor_tensor(out=g, in0=f, in1=a_ap, op=op.is_gt)
                o = sc_tile()
                nc.vector.tensor_tensor(out=o, in0=f, in1=g, op=op.subtract)
                return o

            class MaxAcc:
                def __init__(self, name):
                    self.t = scp.tile([128, 1], dt.float32, name=f"mx_{name}", tag=f"mx_{name}")
                    nc.vector.memset(self.t, 0.0)
                    self.name = name

                def add(self, red_ap, p0=0):
                    p = red_ap.partition_size()
                    nc.vector.tensor_tensor(out=self.t[p0:p0 + p], in0=self.t[p0:p0 + p],
                                            in1=red_ap, op=op.max)

                def add_from(self, src_ap, scratch):
                    nc.vector.tensor_reduce(out=scratch, in_=src_ap, axis=AX.X,
                                            op=op.max, apply_absolute_value=True)
                    self.add(scratch)

                def finish(self):
                    o = scp.tile([1, 1], dt.float32, name=f"mg_{self.name}", tag=f"mg_{self.name}")
                    nc.gpsimd.tensor_reduce(out=o, in_=self.t, axis=AX.C, op=op.max)
                    return o

            scals = sing.tile([1, 8], dt.float32, name="scals")
            nc.sync.dma_start(out=scals, in_=scal_d[:])

            def load_vec_b(dten, n, name):
                t = sing.tile([128, n], dt.float32, name=name)
                nc.sync.dma_start(out=t, in_=bass.AP(tensor=dten, offset=0, ap=[[0, 128], [1, n]]))
                return t

            ln1bi = load_vec_b(ln1bi_d, C, "ln1bi_t")
            ln1sc = load_vec_b(ln1sc_d, C, "ln1sc_t")
            ln2bi = load_vec_b(ln2bi_d, C, "ln2bi_t")
            ln2sc = load_vec_b(ln2sc_d, C, "ln2sc_t")

            def load_cm(dten, nt_, name):
                # [1, nt_*128] channel-major -> [128, nt_] (partition = channel % 128)
                t = sing.tile([128, nt_], dt.float32, name=name)
                nc.sync.dma_start(out=t, in_=bass.AP(tensor=dten, offset=0,
                                                     ap=[[1, 128], [128, nt_]]))
                return t

            wsf_cm = load_cm(wsf_d, CT, "wsf_cm")
            bpw_cm = load_cm(bpw_d, CT, "bpw_cm")
            bfc1_cm = load_cm(bfc1_d, OT_FC1, "bfc1_cm")
            bfc2_cm = load_cm(bfc2_d, CT, "bfc2_cm")

            def floor_cm(src, nt_, rcp_b, name):
                # floor(src * rcp) exact, per-channel [128, nt_]
                x = sing.tile([128, nt_], dt.float32, name=f"{name}_x")
                nc.vector.tensor_scalar(out=x, in0=src, scalar1=rcp_b[:, 0:1], scalar2=None,
                                        op0=op.mult)
                i = sing.tile([128, nt_], dt.int32, name=f"{name}_i")
                nc.vector.tensor_copy(out=i, in_=x)
                f = sing.tile([128, nt_], dt.float32, name=f"{name}_f")
                nc.vector.tensor_copy(out=f, in_=i)
                g = sing.tile([128, nt_], dt.float32, name=f"{name}_g")
                nc.vector.tensor_tensor(out=g, in0=f, in1=x, op=op.is_gt)
                nc.vector.tensor_tensor(out=f, in0=f, in1=g, op=op.subtract)
                return f

            # ---------------- LN stage (token-major, streaming from DRAM) ----------
            def ln_stage(src_d, bias_t, sfc_t, rcp_in_b, tag):
                spill = dr.tile([M, C], dt.float32, name=f"lnsp_{tag}", tag=f"lnsp_{tag}")
                mx = MaxAcc(f"ln_{tag}")
                for it in range(NT):
                    p = min(PT, M - it * PT)
                    xin = tp.tile([128, C], dt.float32, name=f"xin_{tag}_{it}", tag="ln_xin")
                    nc.sync.dma_start(out=xin[:p, :], in_=src_d[it * PT:it * PT + p, :])
                    xint = tp.tile([128, C], dt.float32, name=f"xi_{tag}_{it}", tag="ln_xi")
                    rs = tp.tile([128, 1], dt.float32, name=f"rs_{tag}_{it}", tag="ln_rs")
                    if rcp_in_b is not None:
                        nc.scalar.activation(out=xint[:p, :], in_=xin[:p, :], func=act.Copy,
                                             scale=rcp_in_b[:p, 0:1], accum_out=rs[:p, 0:1])
                    else:
                        nc.scalar.activation(out=xint[:p, :], in_=xin[:p, :], func=act.Copy,
                                             scale=1.0, accum_out=rs[:p, 0:1])
                    mi = tp.tile([128, 1], dt.int32, name=f"mi_{tag}_{it}", tag="ln_mi")
                    mf = tp.tile([128, 1], dt.float32, name=f"mf_{tag}_{it}", tag="ln_mf")
                    nc.vector.tensor_scalar(out=mf[:p], in0=rs[:p], scalar1=float(_f32(1.0 / C)),
                                            scalar2=None, op0=op.mult)
                    nc.vector.tensor_copy(out=mi[:p], in_=mf[:p])
                    nc.vector.tensor_copy(out=mf[:p], in_=mi[:p])
                    y = tp.tile([128, C], dt.float32, name=f"y_{tag}_{it}", tag="ln_y")
                    nc.vector.tensor_scalar(out=y[:p, :], in0=xint[:p, :], scalar1=mf[:p, 0:1],
                                            scalar2=None, op0=op.subtract)
                    sq = tp.tile([128, C], dt.float32, name=f"sq_{tag}_{it}", tag="ln_sq")
                    var = tp.tile([128, 1], dt.float32, name=f"v_{tag}_{it}", tag="ln_v")
                    nc.scalar.activation(out=sq[:p, :], in_=y[:p, :], func=act.Square,
                                         accum_out=var[:p, 0:1])
                    nc.vector.tensor_scalar(out=var[:p], in0=var[:p], scalar1=1.0, scalar2=None,
                                            op0=op.max)
                    std = tp.tile([128, 1], dt.float32, name=f"st_{tag}_{it}", tag="ln_st")
                    nc.scalar.activation(out=std[:p], in_=var[:p], func=act.Sqrt)
                    sti = tp.tile([128, 1], dt.int32, name=f"sti_{tag}_{it}", tag="ln_sti")
                    nc.vector.tensor_scalar(out=std[:p], in0=std[:p], scalar1=0.4999999,
                                            scalar2=None, op0=op.subtract)
                    nc.vector.tensor_copy(out=sti[:p], in_=std[:p])
                    nc.vector.tensor_copy(out=std[:p], in_=sti[:p])
                    rstd = tp.tile([128, 1], dt.float32, name=f"rst_{tag}_{it}", tag="ln_rst")
                    nc.vector.reciprocal(out=rstd[:p], in_=std[:p])
                    fac = tp.tile([128, 1], dt.float32, name=f"fa_{tag}_{it}", tag="ln_fa")
                    nc.vector.tensor_scalar(out=fac[:p], in0=rstd[:p], scalar1=float(2.0 ** 31),
                                            scalar2=0.49, op0=op.mult, op1=op.subtract)
                    fai = tp.tile([128, 1], dt.int32, name=f"fai_{tag}_{it}", tag="ln_fai")
                    nc.vector.tensor_copy(out=fai[:p], in_=fac[:p])
                    nc.vector.tensor_copy(out=fac[:p], in_=fai[:p])
                    nc.vector.tensor_scalar(out=fac[:p], in0=fac[:p], scalar1=0.5, scalar2=None,
                                            op0=op.mult)
                    # floor(y*factor/2) = castRNE(y*(factor/2) - 0.25): args are ints/half-ints
                    nc.vector.tensor_scalar(out=y[:p, :], in0=y[:p, :], scalar1=fac[:p, 0:1],
                                            scalar2=0.25, op0=op.mult, op1=op.subtract)
                    yi = tp.tile([128, C], dt.int32, name=f"yi_{tag}_{it}", tag="ln_yi")
                    nc.vector.tensor_copy(out=yi[:p, :], in_=y[:p, :])
                    nc.vector.tensor_copy(out=y[:p, :], in_=yi[:p, :])
                    nc.vector.tensor_tensor(out=y[:p, :], in0=y[:p, :], in1=bias_t[:p, :],
                                            op=op.add)
                    nc.vector.tensor_tensor(out=y[:p, :], in0=y[:p, :], in1=sfc_t[:p, :],
                                            op=op.mult)
                    red = tp.tile([128, 1], dt.float32, name=f"re_{tag}_{it}", tag="ln_re")
                    mx.add_from(y[:p, :], red[:p])
                    nc.sync.dma_start(out=spill[it * PT:it * PT + p, :], in_=y[:p, :])
                return spill, mx.finish()

            def quant_transpose(spill, rcp_b, dst_tiles, tag):
                """spill [M,C] f32 -> round -> bf16 -> PE transpose -> dst [CT][128, M]"""
                for it in range(NT):
                    p = min(PT, M - it * PT)
                    yb = tp.tile([128, C], dt.float32, name=f"qy_{tag}_{it}", tag="qt_y")
                    nc.sync.dma_start(out=yb[:p, :], in_=spill[it * PT:it * PT + p, :])
                    qi = tp.tile([128, C], dt.int32, name=f"q_{tag}_{it}", tag="qt_qi")
                    nc.scalar.activation(out=qi[:p, :], in_=yb[:p, :], func=act.Copy,
                                         scale=rcp_b[:p, 0:1])
                    qb = tp.tile([128, C], dt.bfloat16, name=f"qb_{tag}_{it}", tag="qt_qb")
                    nc.vector.tensor_copy(out=qb[:p, :], in_=qi[:p, :])
                    for c in range(CT):
                        ps = pt2.tile([128, 128], dt.bfloat16, name=f"pt_{tag}_{it}_{c}",
                                      tag="qt_ps")
                        nc.tensor.transpose(ps[:, :], qb[:, c * PT:(c + 1) * PT], idb[:])
                        nc.scalar.activation(out=dst_tiles[c][:, it * PT:it * PT + p],
                                             in_=ps[:, :p], func=act.Copy)

            # ========================= LN1 + qa1 =========================
            ln1_spill, ln1_lmax = ln_stage(x1_d, ln1bi, ln1sc,
                                           bcast_dram(scal_d, "rcpsf1", 0), "l1")
            g1 = allreduce_max(ln1_lmax, "qa1")
            sf_qa1 = sc_op(g1, op.mult, float(_f32(1.0 / 127.0)))
            sf_qa1_g = sc_op(sf_qa1, op.max, 1e-37)
            rcp_qa1_b = bcast(sc_recip(sf_qa1_g), "rq1")

            with tc.tile_pool(name="mats1", bufs=1) as mats1:
                xqT = [mats1.tile([128, M], dt.bfloat16, name=f"xqT_{c}", tag=f"xqT_{c}")
                       for c in range(CT)]
                quant_transpose(ln1_spill, rcp_qa1_b, xqT, "x1")

                # ===================== QKV matmul -> spill =====================
                qkv_spill = dr.tile([3 * C, M], dt.float32, name="qkv_spill")
                mx_qkv = MaxAcc("qkv")
                for ot in range(OT_QKV):
                    wts = []
                    for kt in range(CT):
                        w = wp.tile([128, 128], dt.bfloat16, name=f"wq_{ot}_{kt}", tag="wq")
                        nc.sync.dma_start(out=w, in_=wqkvT_d[kt * PT:(kt + 1) * PT,
                                                            ot * PT:(ot + 1) * PT])
                        wts.append(w)
                    for (c0, cw) in CHUNKS:
                        ps = pp.tile([128, CHUNK], dt.float32, name=f"pq_{ot}_{c0}", tag="pq")
                        for kt in range(CT):
                            nc.tensor.matmul(ps[:, :cw], wts[kt][:, :], xqT[kt][:, c0:c0 + cw],
                                             start=(kt == 0), stop=(kt == CT - 1))
                        sb = tp.tile([128, CHUNK], dt.float32, name=f"sq_{ot}_{c0}", tag="mm_sb")
                        nc.scalar.activation(out=sb[:, :cw], in_=ps[:, :cw], func=act.Copy)
                        red = tp.tile([128, 1], dt.float32, name=f"rq_{ot}_{c0}", tag="mm_red")
                        mx_qkv.add_from(sb[:, :cw], red)
                        nc.sync.dma_start(out=qkv_spill[ot * PT:(ot + 1) * PT, c0:c0 + cw],
                                          in_=sb[:, :cw])
                qkv_lmax = mx_qkv.finish()
            g2 = allreduce_max(qkv_lmax, "qa2")
            out_sf_qkv = sc_mul2(sf_qa1, scals[0:1, 1:2])
            sf1a = sc_op(sc_mul2(g2, out_sf_qkv), op.mult, float(_f32(1.0 / 127.0)))
            sf1a_g = sc_op(sf1a, op.max, 1e-37)
            ratio_qkv_b = bcast(sc_mul2(out_sf_qkv, sc_recip(sf1a_g)), "rqkv")

            with tc.tile_pool(name="qkp", bufs=1) as qkp:
                qkvT = [qkp.tile([128, M], dt.bfloat16, name=f"qkvT_{t}", tag=f"qkvT_{t}")
                        for t in range(OT_QKV)]
                for ot in range(OT_QKV):
                    for (c0, cw) in CHUNKS:
                        sb = tp.tile([128, CHUNK], dt.float32, name=f"uq_{ot}_{c0}", tag="mm_u")
                        nc.sync.dma_start(out=sb[:, :cw],
                                          in_=qkv_spill[ot * PT:(ot + 1) * PT, c0:c0 + cw])
                        qi = tp.tile([128, CHUNK], dt.int32, name=f"uqi_{ot}_{c0}", tag="mm_ui")
                        nc.scalar.activation(out=qi[:, :cw], in_=sb[:, :cw], func=act.Copy,
                                             scale=ratio_qkv_b[:, 0:1])
                        nc.vector.tensor_copy(out=qkvT[ot][:, c0:c0 + cw], in_=qi[:, :cw])

                def qT_ap(h, b):
                    return qkvT[h // 2][(h % 2) * 64:(h % 2) * 64 + 64, b * N:(b + 1) * N]

                def kT_ap(h, b):
                    return qkvT[CT + h // 2][(h % 2) * 64:(h % 2) * 64 + 64, b * N:(b + 1) * N]

                def vT_ap(h, b):
                    return qkvT[2 * CT + h // 2][(h % 2) * 64:(h % 2) * 64 + 64,
                                                 b * N:(b + 1) * N]

                NS = [(0, 128), (128, 69)]
                # ----- scores pass 1: absmax only -----
                mx_s = MaxAcc("scores")
                for b in range(PER):
                    for h in range(H):
                        for (n0, nw) in NS:
                            ps = pss.tile([128, N], dt.float32, name=f"ps_{b}_{h}_{n0}",
                                          tag="ps_s")
                            nc.tensor.matmul(ps[:nw, :], qT_ap(h, b)[:, n0:n0 + nw], kT_ap(h, b),
                                             start=True, stop=True)
                            red = tp.tile([128, 1], dt.float32, name=f"rs_{b}_{h}_{n0}",
                                          tag="mm_red")
                            mx_s.add_from(ps[:nw, :], red[:nw])
                s_lmax = mx_s.finish()
                g3 = allreduce_max(s_lmax, "qa3")
                sfa = sc_op(sc_mul2(sf1a, sf1a), op.mult, float(_f32(D ** -0.5)))
                sf_s = sc_op(sc_mul2(g3, sfa), op.mult, float(_f32(1.0 / 127.0)))
                sf_s_g = sc_op(sf_s, op.max, 1e-37)
                rcp_sf_s = sc_recip(sf_s_g)
                ratio_s_b = bcast(sc_mul2(sfa, rcp_sf_s), "rs")
                x0i = sc_floor(sc_op(rcp_sf_s, op.mult, -0.6931))
                bi_s = sc_floor(sc_op(rcp_sf_s, op.mult, float(_f32(0.96963238 / 0.35815147))))
                ci_s = sc_floor(sc_op(sc_mul2(rcp_sf_s, rcp_sf_s), op.mult,
                                      float(_f32(1.0 / 0.35815147))))
                clamp_b = bcast(sc_op(x0i, op.mult, 30.0), "clmp")
                rcpx0_b = bcast(sc_recip(x0i), "rcpx0")
                negx0_b = bcast(sc_op(x0i, op.mult, -1.0), "negx0")
                bi_b = bcast(bi_s, "bis")
                ci_b = bcast(ci_s, "cis")

                av_spill = dr.tile([C, M], dt.float32, name="av_spill")
                mx_av = MaxAcc("av")
                with tc.tile_pool(name="smp", bufs=3) as smp:
                    for b in range(PER):
                        for h in range(H):
                            r0v = (h % 2) * 64
                            vtok = []
                            for (n0, nw) in NS:
                                pv = pt2.tile([128, 64], dt.bfloat16, name=f"pv_{b}_{h}_{n0}",
                                              tag="pv")
                                nc.tensor.transpose(pv[:nw, :], vT_ap(h, b)[:, n0:n0 + nw],
                                                    idb[r0v:r0v + 64, r0v:r0v + 64])
                                vt = smp.tile([128, 64], dt.float32, name=f"vt_{b}_{h}_{n0}",
                                              tag=f"vt_{n0}")
                                nc.scalar.activation(out=vt[:nw, :], in_=pv[:nw, :],
                                                     func=act.Copy)
                                vtok.append(vt)
                            at_parts = {}
                            for (n0, nw) in NS:
                                ps = pss.tile([128, N], dt.float32, name=f"p2_{b}_{h}_{n0}",
                                              tag="ps_s")
                                nc.tensor.matmul(ps[:nw, :], qT_ap(h, b)[:, n0:n0 + nw],
                                                 kT_ap(h, b), start=True, stop=True)
                                xi = smp.tile([128, N], dt.int32, name=f"sxi_{b}_{h}_{n0}",
                                              tag="sm_xi")
                                nc.scalar.activation(out=xi[:nw, :], in_=ps[:nw, :],
                                                     func=act.Copy, scale=ratio_s_b[:nw, 0:1])
                                x = smp.tile([128, N], dt.float32, name=f"sx_{b}_{h}_{n0}",
                                             tag="sm_x")
                                nc.vector.tensor_copy(out=x[:nw, :], in_=xi[:nw, :])
                                rm = smp.tile([128, 1], dt.float32, name=f"srm_{b}_{h}_{n0}",
                                              tag="sm_rm")
                                nc.vector.tensor_reduce(out=rm[:nw], in_=x[:nw, :], axis=AX.X,
                                                        op=op.max)
                                nc.vector.tensor_scalar(out=x[:nw, :], in0=x[:nw, :],
                                                        scalar1=rm[:nw, 0:1],
                                                        scalar2=clamp_b[:nw, 0:1],
                                                        op0=op.subtract, op1=op.max)
                                qf = smp.tile([128, N], dt.float32, name=f"sqf_{b}_{h}_{n0}",
                                              tag="sm_qf")
                                nc.vector.tensor_scalar(out=qf[:nw, :], in0=x[:nw, :],
                                                        scalar1=rcpx0_b[:nw, 0:1], scalar2=0.49,
                                                        op0=op.mult, op1=op.subtract)
                                qi32 = smp.tile([128, N], dt.int32, name=f"sqi_{b}_{h}_{n0}",
                                                tag="sm_qi")
                                nc.vector.tensor_copy(out=qi32[:nw, :], in_=qf[:nw, :])
                                nc.vector.tensor_copy(out=qf[:nw, :], in_=qi32[:nw, :])
                                r = smp.tile([128, N], dt.float32, name=f"sr_{b}_{h}_{n0}",
                                             tag="sm_r")
                                nc.vector.scalar_tensor_tensor(out=r[:nw, :], in0=qf[:nw, :],
                                                               scalar=negx0_b[:nw, 0:1],
                                                               in1=x[:nw, :], op0=op.mult,
                                                               op1=op.add)
                                t = smp.tile([128, N], dt.float32, name=f"stp_{b}_{h}_{n0}",
                                             tag="sm_t")
                                nc.vector.scalar_tensor_tensor(out=t[:nw, :], in0=r[:nw, :],
                                                               scalar=bi_b[:nw, 0:1],
                                                               in1=r[:nw, :], op0=op.add,
                                                               op1=op.mult)
                                ei = smp.tile([128, N], dt.int32, name=f"sei_{b}_{h}_{n0}",
                                              tag="sm_ei")
                                nc.vector.tensor_scalar(out=ei[:nw, :], in0=qi32[:nw, :],
                                                        scalar1=-1, scalar2=157, op0=op.mult,
                                                        op1=op.add)
                                nc.vector.tensor_scalar(out=ei[:nw, :], in0=ei[:nw, :],
                                                        scalar1=23, scalar2=None,
                                                        op0=op.logical_shift_left)
                                ex = smp.tile([128, N], dt.float32, name=f"sex_{b}_{h}_{n0}",
                                              tag="sm_ex")
                                nc.vector.scalar_tensor_tensor(
                                    out=ex[:nw, :], in0=t[:nw, :], scalar=ci_b[:nw, 0:1],
                                    in1=ei[:nw, :].bitcast(dt.float32), op0=op.add, op1=op.mult)
                                rsum = smp.tile([128, 1], dt.float32, name=f"ssu_{b}_{h}_{n0}",
                                                tag="sm_su")
                                nc.vector.tensor_scalar(out=ex[:nw, :], in0=ex[:nw, :],
                                                        scalar1=0.0, scalar2=None, op0=op.max,
                                                        op1=op.add, accum_out=rsum[:nw, 0:1])
                                rp = smp.tile([128, 1], dt.float32, name=f"srp_{b}_{h}_{n0}",
                                              tag="sm_rp")
                                nc.vector.reciprocal(out=rp[:nw], in_=rsum[:nw])
                                nc.vector.tensor_scalar(out=rp[:nw], in0=rp[:nw],
                                                        scalar1=float(2.0 ** 32), scalar2=0.49,
                                                        op0=op.mult, op1=op.subtract)
                                rpi = smp.tile([128, 1], dt.int32, name=f"srpi_{b}_{h}_{n0}",
                                               tag="sm_rpi")
                                nc.vector.tensor_copy(out=rpi[:nw], in_=rp[:nw])
                                nc.vector.tensor_copy(out=rp[:nw], in_=rpi[:nw])
                                nc.vector.tensor_scalar(out=rp[:nw], in0=rp[:nw],
                                                        scalar1=float(2.0 ** -16), scalar2=None,
                                                        op0=op.mult)
                                nc.vector.tensor_scalar(out=ex[:nw, :], in0=ex[:nw, :],
                                                        scalar1=rp[:nw, 0:1], scalar2=0.49,
                                                        op0=op.mult, op1=op.subtract)
                                exi = smp.tile([128, N], dt.int32, name=f"sxe_{b}_{h}_{n0}",
                                               tag="sm_xe")
                                nc.vector.tensor_copy(out=exi[:nw, :], in_=ex[:nw, :])
                                nc.vector.tensor_copy(out=ex[:nw, :], in_=exi[:nw, :])
                                for (m0, mw) in NS:
                                    pa = pt2.tile([128, 128], dt.float32,
                                                  name=f"pa_{b}_{h}_{n0}_{m0}", tag="pa")
                                    nc.tensor.transpose(pa[:mw, :nw], ex[:nw, m0:m0 + mw],
                                                        idf[:nw, :nw])
                                    at = smp.tile([128, 128], dt.float32,
                                                  name=f"at_{b}_{h}_{n0}_{m0}",
                                                  tag=f"at_{m0}_{n0}")
                                    nc.scalar.activation(out=at[:mw, :nw], in_=pa[:mw, :nw],
                                                         func=act.Copy)
                                    at_parts[(m0, n0)] = at
                            pav = pt2.tile([64, N], dt.float32, name=f"pav_{b}_{h}", tag="pav")
                            for mi, (m0, mw) in enumerate(NS):
                                rhs = smp.tile([128, N], dt.float32, name=f"rhs_{b}_{h}_{m0}",
                                               tag=f"rhs_{m0}")
                                for (n0, nw) in NS:
                                    nc.vector.tensor_copy(out=rhs[:mw, n0:n0 + nw],
                                                          in_=at_parts[(m0, n0)][:mw, :nw])
                                nc.tensor.matmul(pav[:, :], vtok[mi][:mw, :], rhs[:mw, :],
                                                 start=(mi == 0), stop=(mi == 1))
                            ov = smp.tile([64, N], dt.float32, name=f"ov_{b}_{h}", tag="ov")
                            nc.scalar.activation(out=ov[:, :], in_=pav[:, :], func=act.Copy)
                            nc.sync.dma_start(
                                out=av_spill[h * 64:(h + 1) * 64, b * N:(b + 1) * N], in_=ov)
                            red = smp.tile([64, 1], dt.float32, name=f"rav_{b}_{h}", tag="rav")
                            nc.vector.tensor_reduce(out=red[:64], in_=pav[:, :], axis=AX.X,
                                                    op=op.max, apply_absolute_value=True)
                            mx_av.add(red[:64])
            av_lmax = mx_av.finish()
            g4 = allreduce_max(av_lmax, "qa4")   # 0 in the realistic case
            sf_av_in = sc_op(sf1a_g, op.mult, float(2.0 ** -16))
            sf_av = sc_op(sc_mul2(g4, sf_av_in), op.mult, float(_f32(1.0 / 127.0)))
            sf_av_g = sc_op(sf_av, op.max, 1e-37)
            rcp_av = sc_recip(sf_av_g)
            ratio_av_b = bcast(sc_mul2(sf_av_in, rcp_av), "ravb")

            with tc.tile_pool(name="aqp", bufs=1) as aqp:
                attn_q = [aqp.tile([128, M], dt.bfloat16, name=f"aq_{c}", tag=f"aq_{c}")
                          for c in range(CT)]
                for c in range(CT):
                    for (c0, cw) in CHUNKS:
                        sb = tp.tile([128, CHUNK], dt.float32, name=f"aqs_{c}_{c0}", tag="mm_u")
                        nc.sync.dma_start(out=sb[:, :cw],
                                          in_=av_spill[c * PT:(c + 1) * PT, c0:c0 + cw])
                        qi = tp.tile([128, CHUNK], dt.int32, name=f"aqi_{c}_{c0}", tag="mm_ui")
                        nc.scalar.activation(out=qi[:, :cw], in_=sb[:, :cw], func=act.Copy,
                                             scale=ratio_av_b[:, 0:1])
                        nc.vector.tensor_copy(out=attn_q[c][:, c0:c0 + cw], in_=qi[:, :cw])

                # ---- proj ----
                bip = floor_cm(bpw_cm, CT, bcast(rcp_av, "rav2"), "bip")
                pj_spill = dr.tile([C, M], dt.float32, name="pj_spill")
                mx_pj = MaxAcc("proj")
                for otc in range(CT):
                    wts = []
                    for kt in range(CT):
                        w = wp.tile([128, 128], dt.bfloat16, name=f"wpj_{otc}_{kt}", tag="wq")
                        nc.sync.dma_start(out=w, in_=wprojT_d[kt * PT:(kt + 1) * PT,
                                                             otc * PT:(otc + 1) * PT])
                        wts.append(w)
                    for (c0, cw) in CHUNKS:
                        ps = pp.tile([128, CHUNK], dt.float32, name=f"ppj_{otc}_{c0}", tag="pq")
                        for kt in range(CT):
                            nc.tensor.matmul(ps[:, :cw], wts[kt][:, :],
                                             attn_q[kt][:, c0:c0 + cw],
                                             start=(kt == 0), stop=(kt == CT - 1))
                        sb = tp.tile([128, CHUNK], dt.float32, name=f"spj_{otc}_{c0}",
                                     tag="mm_sb")
                        nc.vector.tensor_scalar(out=sb[:, :cw], in0=ps[:, :cw],
                                                scalar1=bip[:, otc:otc + 1], scalar2=None,
                                                op0=op.add)
                        red = tp.tile([128, 1], dt.float32, name=f"rpj_{otc}_{c0}", tag="mm_red")
                        nc.vector.tensor_reduce(out=red, in_=sb[:, :cw], axis=AX.X, op=op.max,
                                                apply_absolute_value=True)
                        nc.vector.tensor_tensor(out=red, in0=red, in1=wsf_cm[:, otc:otc + 1],
                                                op=op.mult)
                        mx_pj.add(red)
                        nc.sync.dma_start(out=pj_spill[otc * PT:(otc + 1) * PT, c0:c0 + cw],
                                          in_=sb[:, :cw])
                pj_lmax = mx_pj.finish()
            g5 = allreduce_max(pj_lmax, "qa5")
            sf5 = sc_op(sc_mul2(g5, sf_av_g), op.mult, float(_f32(1.0 / 32767.0)))
            sf5_g = sc_op(sf5, op.max, 1e-37)
            t_r5 = sing.tile([128, CT], dt.float32, name="t_r5")
            nc.vector.tensor_scalar(out=t_r5, in0=wsf_cm,
                                    scalar1=bcast(sf_av_g, "sav3")[:, 0:1],
                                    scalar2=bcast(sc_recip(sf5_g), "r5b")[:, 0:1],
                                    op0=op.mult, op1=op.mult)

            # quantize proj, transpose to token-major, resid1
            r1_spill = dr.tile([M, C], dt.float32, name="r1_spill")
            sf5_b = bcast(sf5_g, "sf5b")
            mx_r1 = MaxAcc("r1")
            for it in range(NT):
                p = min(PT, M - it * PT)
                ytok = tp.tile([128, C], dt.float32, name=f"ytk_{it}", tag="ytk")
                for c in range(CT):
                    sb = tp.tile([128, 128], dt.float32, name=f"pjl_{it}_{c}", tag="pjl")
                    nc.sync.dma_start(out=sb[:, :p], in_=pj_spill[c * PT:(c + 1) * PT,
                                                                  it * PT:it * PT + p])
                    qi = tp.tile([128, 128], dt.int32, name=f"pji_{it}_{c}", tag="pji")
                    nc.scalar.activation(out=qi[:, :p], in_=sb[:, :p], func=act.Copy,
                                         scale=t_r5[:, c:c + 1])
                    qf = tp.tile([128, 128], dt.float32, name=f"pjf_{it}_{c}", tag="pjf")
                    nc.vector.tensor_copy(out=qf[:, :p], in_=qi[:, :p])
                    if p < 128:
                        nc.vector.memset(qf[:, p:], 0.0)
                    pa = pt2.tile([128, 128], dt.float32, name=f"pjt_{it}_{c}", tag="pa")
                    nc.tensor.transpose(pa[:, :], qf[:, :], idf[:])
                    nc.scalar.activation(out=ytok[:p, c * PT:(c + 1) * PT], in_=pa[:p, :],
                                         func=act.Copy)
                x1t = tp.tile([128, C], dt.float32, name=f"x1r_{it}", tag="ln_xin")
                nc.sync.dma_start(out=x1t[:p, :], in_=x1_d[it * PT:it * PT + p, :])
                nc.vector.scalar_tensor_tensor(out=ytok[:p, :], in0=ytok[:p, :],
                                               scalar=sf5_b[:p, 0:1], in1=x1t[:p, :],
                                               op0=op.mult, op1=op.add)
                red = tp.tile([128, 1], dt.float32, name=f"rr1_{it}", tag="mm_red")
                mx_r1.add_from(ytok[:p, :], red[:p])
                nc.sync.dma_start(out=r1_spill[it * PT:it * PT + p, :], in_=ytok[:p, :])
            r1_lmax = mx_r1.finish()
            g6 = allreduce_max(r1_lmax, "qa6")
            sf2 = sc_op(g6, op.mult, float(_f32(1.0 / 32767.0)))
            sf2_g = sc_op(sf2, op.max, 1e-37)
            rcp2_b = bcast(sc_recip(sf2_g), "rcp2")
            x2q_spill = dr.tile([M, C], dt.float32, name="x2q_spill")
            for it in range(NT):
                p = min(PT, M - it * PT)
                yb = tp.tile([128, C], dt.float32, name=f"x2l_{it}", tag="qt_y")
                nc.sync.dma_start(out=yb[:p, :], in_=r1_spill[it * PT:it * PT + p, :])
                qi = tp.tile([128, C], dt.int32, name=f"x2i_{it}", tag="qt_qi")
                nc.scalar.activation(out=qi[:p, :], in_=yb[:p, :], func=act.Copy,
                                     scale=rcp2_b[:p, 0:1])
                qf = tp.tile([128, C], dt.float32, name=f"x2f_{it}", tag="x2f")
                nc.vector.tensor_copy(out=qf[:p, :], in_=qi[:p, :])
                nc.sync.dma_start(out=x2q_spill[it * PT:it * PT + p, :], in_=qf[:p, :])

            # ========================= LN2 + qa7 =========================
            ln2_spill, ln2_lmax = ln_stage(x2q_spill, ln2bi, ln2sc, None, "l2")
            g7 = allreduce_max(ln2_lmax, "qa7")
            sf7 = sc_op(g7, op.mult, float(_f32(1.0 / 127.0)))
            sf7_g = sc_op(sf7, op.max, 1e-37)
            rcp7_b = bcast(sc_recip(sf7_g), "rcp7")

            out_sf_fc1 = sc_mul2(sf7, scals[0:1, 2:3])
            out_sf_fc1_g = sc_op(out_sf_fc1, op.max, 1e-37)

            with tc.tile_pool(name="mats2", bufs=1) as mats2:
                xqT2 = [mats2.tile([128, M], dt.bfloat16, name=f"xqT2_{c}", tag=f"xqT2_{c}")
                        for c in range(CT)]
                quant_transpose(ln2_spill, rcp7_b, xqT2, "x2")

                # ===================== FC1 =====================
                bf1 = floor_cm(bfc1_cm, OT_FC1, bcast(sc_recip(out_sf_fc1_g), "rosf1"), "bf1")
                fc1_spill = dr.tile([HID, M], dt.float32, name="fc1_spill")
                mx_f1 = MaxAcc("fc1")
                for ot in range(OT_FC1):
                    wts = []
                    for kt in range(CT):
                        w = wp.tile([128, 128], dt.bfloat16, name=f"wf1_{ot}_{kt}", tag="wq")
                        nc.sync.dma_start(out=w, in_=wfc1T_d[kt * PT:(kt + 1) * PT,
                                                            ot * PT:(ot + 1) * PT])
                        wts.append(w)
                    for (c0, cw) in CHUNKS:
                        ps = pp.tile([128, CHUNK], dt.float32, name=f"pf1_{ot}_{c0}", tag="pq")
                        for kt in range(CT):
                            nc.tensor.matmul(ps[:, :cw], wts[kt][:, :], xqT2[kt][:, c0:c0 + cw],
                                             start=(kt == 0), stop=(kt == CT - 1))
                        sb = tp.tile([128, CHUNK], dt.float32, name=f"sf1_{ot}_{c0}",
                                     tag="mm_sb")
                        nc.vector.tensor_scalar(out=sb[:, :cw], in0=ps[:, :cw],
                                                scalar1=bf1[:, ot:ot + 1], scalar2=None,
                                                op0=op.add)
                        red = tp.tile([128, 1], dt.float32, name=f"rf1_{ot}_{c0}", tag="mm_red")
                        mx_f1.add_from(sb[:, :cw], red)
                        nc.sync.dma_start(out=fc1_spill[ot * PT:(ot + 1) * PT, c0:c0 + cw],
                                          in_=sb[:, :cw])
                f1_lmax = mx_f1.finish()
            g8 = allreduce_max(f1_lmax, "qa8")
            sf8 = sc_op(sc_mul2(g8, out_sf_fc1_g), op.mult, float(_f32(1.0 / 127.0)))
            sf8_g = sc_op(sf8, op.max, 1e-37)
            ratio_f1_b = bcast(sc_mul2(out_sf_fc1_g, sc_recip(sf8_g)), "rf1b")

            rcp_e = sc_op(sc_recip(sf8_g), op.mult, 1.4142)
            bi_g = sc_floor(sc_op(rcp_e, op.mult, -1.769))
            nbi_g_b = bcast(sc_op(bi_g, op.mult, -1.0), "nbig")
            bi_g_b = bcast(bi_g, "big")
            ci_gf = sc_op(sc_mul2(rcp_e, rcp_e), op.mult, float(_f32(1.0 / -0.2888)))
            ci_g = sc_floor(ci_gf)
            ci_g_b = bcast(ci_g, "cig")
            shift_b = ci_g_b  # floor(1/erf_sf) == floor(ci_gf) == ci_g

            gelu_spill = dr.tile([HID, M], dt.float32, name="gelu_spill")
            mx_ge = MaxAcc("gelu")
            with tc.tile_pool(name="gep", bufs=2) as gep:
                for ot in range(OT_FC1):
                    for (c0, cw) in CHUNKS:
                        sb = tp.tile([128, CHUNK], dt.float32, name=f"gi_{ot}_{c0}", tag="mm_u")
                        nc.sync.dma_start(out=sb[:, :cw],
                                          in_=fc1_spill[ot * PT:(ot + 1) * PT, c0:c0 + cw])
                        qi = tp.tile([128, CHUNK], dt.int32, name=f"gqi_{ot}_{c0}", tag="mm_ui")
                        nc.scalar.activation(out=qi[:, :cw], in_=sb[:, :cw], func=act.Copy,
                                             scale=ratio_f1_b[:, 0:1])
                        xg = gep.tile([128, CHUNK], dt.float32, name=f"gx_{ot}_{c0}", tag="gx")
                        nc.vector.tensor_copy(out=xg[:, :cw], in_=qi[:, :cw])
                        sg = gep.tile([128, CHUNK], dt.float32, name=f"gs_{ot}_{c0}", tag="gs")
                        nc.scalar.activation(out=sg[:, :cw], in_=xg[:, :cw], func=act.Sign)
                        ab = gep.tile([128, CHUNK], dt.float32, name=f"ga_{ot}_{c0}", tag="ga")
                        nc.scalar.activation(out=ab[:, :cw], in_=xg[:, :cw], func=act.Abs)
                        nc.vector.tensor_scalar(out=ab[:, :cw], in0=ab[:, :cw],
                                                scalar1=nbi_g_b[:, 0:1],
                                                scalar2=bi_g_b[:, 0:1],
                                                op0=op.min, op1=op.add)
                        sq = gep.tile([128, CHUNK], dt.float32, name=f"gq2_{ot}_{c0}", tag="gq2")
                        nc.scalar.activation(out=sq[:, :cw], in_=ab[:, :cw], func=act.Square)
                        nc.vector.scalar_tensor_tensor(out=sq[:, :cw], in0=sq[:, :cw],
                                                       scalar=ci_g_b[:, 0:1], in1=sg[:, :cw],
                                                       op0=op.add, op1=op.mult)
                        nc.vector.scalar_tensor_tensor(out=xg[:, :cw], in0=sq[:, :cw],
                                                       scalar=shift_b[:, 0:1], in1=xg[:, :cw],
                                                       op0=op.add, op1=op.mult)
                        red = tp.tile([128, 1], dt.float32, name=f"rge_{ot}_{c0}", tag="mm_red")
                        mx_ge.add_from(xg[:, :cw], red)
                        nc.sync.dma_start(out=gelu_spill[ot * PT:(ot + 1) * PT, c0:c0 + cw],
                                          in_=xg[:, :cw])
            ge_lmax = mx_ge.finish()
            g9 = allreduce_max(ge_lmax, "qa9")
            sf_ge_out = sc_op(sc_mul2(sf8_g, sc_recip(ci_gf)), op.mult, 0.5)  # negative
            sf9 = sc_op(sc_mul2(g9, sc_op(sf_ge_out, op.mult, -1.0)), op.mult,
                        float(_f32(1.0 / 127.0)))
            sf9_g = sc_op(sf9, op.max, 1e-37)
            ratio_ge_b = bcast(sc_mul2(sf_ge_out, sc_recip(sf9_g)), "rgeb")

            with tc.tile_pool(name="geq", bufs=1) as geqp:
                xq_ge = [geqp.tile([128, M], dt.bfloat16, name=f"xge_{t}", tag=f"xge_{t}")
                         for t in range(OT_FC1)]
                for ot in range(OT_FC1):
                    for (c0, cw) in CHUNKS:
                        sb = tp.tile([128, CHUNK], dt.float32, name=f"ge2_{ot}_{c0}", tag="mm_u")
                        nc.sync.dma_start(out=sb[:, :cw],
                                          in_=gelu_spill[ot * PT:(ot + 1) * PT, c0:c0 + cw])
                        qi = tp.tile([128, CHUNK], dt.int32, name=f"ge2i_{ot}_{c0}",
                                     tag="mm_ui")
                        nc.scalar.activation(out=qi[:, :cw], in_=sb[:, :cw], func=act.Copy,
                                             scale=ratio_ge_b[:, 0:1])
                        nc.vector.tensor_copy(out=xq_ge[ot][:, c0:c0 + cw], in_=qi[:, :cw])

                # ===================== FC2 =====================
                out_sf_fc2 = sc_mul2(sf9, scals[0:1, 3:4])
                out_sf_fc2_g = sc_op(out_sf_fc2, op.max, 1e-37)
                bf2 = floor_cm(bfc2_cm, CT, bcast(sc_recip(out_sf_fc2_g), "rosf2"), "bf2")
                f2_spill = dr.tile([C, M], dt.float32, name="f2_spill")
                mx_f2 = MaxAcc("fc2")
                for otc in range(CT):
                    wts = []
                    for kt in range(OT_FC1):
                        w = wp2.tile([128, 128], dt.bfloat16, name=f"wf2_{otc}_{kt}", tag="wq2")
                        nc.sync.dma_start(out=w, in_=wfc2T_d[kt * PT:(kt + 1) * PT,
                                                            otc * PT:(otc + 1) * PT])
                        wts.append(w)
                    for (c0, cw) in CHUNKS:
                        ps = pp.tile([128, CHUNK], dt.float32, name=f"pf2_{otc}_{c0}", tag="pq")
                        for kt in range(OT_FC1):
                            nc.tensor.matmul(ps[:, :cw], wts[kt][:, :],
                                             xq_ge[kt][:, c0:c0 + cw],
                                             start=(kt == 0), stop=(kt == OT_FC1 - 1))
                        sb = tp.tile([128, CHUNK], dt.float32, name=f"sf2_{otc}_{c0}",
                                     tag="mm_sb")
                        nc.vector.tensor_scalar(out=sb[:, :cw], in0=ps[:, :cw],
                                                scalar1=bf2[:, otc:otc + 1], scalar2=None,
                                                op0=op.add)
                        red = tp.tile([128, 1], dt.float32, name=f"rf2_{otc}_{c0}",
                                      tag="mm_red")
                        mx_f2.add_from(sb[:, :cw], red)
                        nc.sync.dma_start(out=f2_spill[otc * PT:(otc + 1) * PT, c0:c0 + cw],
                                          in_=sb[:, :cw])
                f2_lmax = mx_f2.finish()
            g10 = allreduce_max(f2_lmax, "qa10")
            sf10 = sc_op(sc_mul2(g10, out_sf_fc2_g), op.mult, float(_f32(1.0 / 32767.0)))
            sf10_g = sc_op(sf10, op.max, 1e-37)
            ratio_f2_b = bcast(sc_mul2(out_sf_fc2_g, sc_recip(sf10_g)), "rf2b")

            r2_spill = dr.tile([M, C], dt.float32, name="r2_spill")
            sf10_b = bcast(sf10_g, "s10b")
            sf2_b2 = bcast(sf2_g, "s2b2")
            mx_r2 = MaxAcc("r2")
            for it in range(NT):
                p = min(PT, M - it * PT)
                ytok = tp.tile([128, C], dt.float32, name=f"y2tk_{it}", tag="ytk")
                for c in range(CT):
                    sb = tp.tile([128, 128], dt.float32, name=f"f2l_{it}_{c}", tag="pjl")
                    nc.sync.dma_start(out=sb[:, :p], in_=f2_spill[c * PT:(c + 1) * PT,
                                                                  it * PT:it * PT + p])
                    qi = tp.tile([128, 128], dt.int32, name=f"f2i_{it}_{c}", tag="pji")
                    nc.scalar.activation(out=qi[:, :p], in_=sb[:, :p], func=act.Copy,
                                         scale=ratio_f2_b[:, 0:1])
                    qf = tp.tile([128, 128], dt.float32, name=f"f2f_{it}_{c}", tag="pjf")
                    nc.vector.tensor_copy(out=qf[:, :p], in_=qi[:, :p])
                    if p < 128:
                        nc.vector.memset(qf[:, p:], 0.0)
                    pa = pt2.tile([128, 128], dt.float32, name=f"f2t_{it}_{c}", tag="pa")
                    nc.tensor.transpose(pa[:, :], qf[:, :], idf[:])
                    nc.scalar.activation(out=ytok[:p, c * PT:(c + 1) * PT], in_=pa[:p, :],
                                         func=act.Copy)
                x2t = tp.tile([128, C], dt.float32, name=f"x2r_{it}", tag="ln_xin")
                nc.sync.dma_start(out=x2t[:p, :], in_=x2q_spill[it * PT:it * PT + p, :])
                nc.vector.tensor_scalar(out=x2t[:p, :], in0=x2t[:p, :],
                                        scalar1=sf2_b2[:p, 0:1], scalar2=None, op0=op.mult)
                nc.vector.scalar_tensor_tensor(out=ytok[:p, :], in0=ytok[:p, :],
                                               scalar=sf10_b[:p, 0:1], in1=x2t[:p, :],
                                               op0=op.mult, op1=op.add)
                red = tp.tile([128, 1], dt.float32, name=f"rr2_{it}", tag="mm_red")
                mx_r2.add_from(ytok[:p, :], red[:p])
                nc.sync.dma_start(out=r2_spill[it * PT:it * PT + p, :], in_=ytok[:p, :])
            r2_lmax = mx_r2.finish()
            g11 = allreduce_max(r2_lmax, "qa11")
            sff = sc_op(g11, op.mult, float(_f32(1.0 / 32767.0)))
            sff_g = sc_op(sff, op.max, 1e-37)
            rcpf_b = bcast(sc_recip(sff_g), "rcpf")
            sff_b = bcast(sff_g, "sffb")
            for it in range(NT):
                p = min(PT, M - it * PT)
                yb = tp.tile([128, C], dt.float32, name=f"fol_{it}", tag="qt_y")
                nc.sync.dma_start(out=yb[:p, :], in_=r2_spill[it * PT:it * PT + p, :])
                qi = tp.tile([128, C], dt.int32, name=f"fo_{it}", tag="qt_qi")
                nc.scalar.activation(out=qi[:p, :], in_=yb[:p, :], func=act.Copy,
                                     scale=rcpf_b[:p, 0:1])
                fo = tp.tile([128, C], dt.float32, name=f"fof_{it}", tag="x2f")
                nc.vector.tensor_copy(out=fo[:p, :], in_=qi[:p, :])
                nc.vector.tensor_scalar(out=fo[:p, :], in0=fo[:p, :], scalar1=sff_b[:p, 0:1],
                                        scalar2=None, op0=op.mult)
                nc.sync.dma_start(out=out_d[it * PT:it * PT + p, :], in_=fo[:p, :])

            st = sing.tile([1, 16], dt.float32, name="st")
            nc.vector.memset(st, 0.0)
            for i, s in enumerate([g4, sff, sf_qa1, sf1a, sf_s, sf2, sf5, sf7, sf9, sf10]):
                nc.vector.tensor_copy(out=st[0:1, i:i + 1], in_=s[0:1, 0:1])
            nc.sync.dma_start(out=stats_d[:], in_=st)

    nc.finalize()
    return nc


def _get_module():
    global _BUILT
    if _BUILT is None:
        _BUILT = _build_module()
    return _BUILT


_TRACE = False
_LAST_RESULT = [None]


def kernel(x_1, act_scaling_factor_1, ln1_w, ln1_b, w_qkv, w_proj, b_proj,
           ln2_w, ln2_b, w_fc1, b_fc1, w_fc2, b_fc2):
    x_1 = np.asarray(x_1, np.float32)
    sf1 = _f32(np.asarray(act_scaling_factor_1).reshape(()))

    wq_ter, alpha_q = _ternarize(w_qkv)
    wf1_ter, alpha_1 = _ternarize(w_fc1)
    wf2_ter, alpha_2 = _ternarize(w_fc2)
    wp_int, wp_sf = _quant_w(w_proj)

    to_bf = lambda a: np.ascontiguousarray(a.T).astype(ml_dtypes.bfloat16)
    wqkvT = to_bf(wq_ter)
    wprojT = to_bf(wp_int)
    wfc1T = to_bf(wf1_ter)
    wfc2T = to_bf(wf2_ter)

    sfp = _f32(math.sqrt(C)) / _f32(2.0 ** 30)
    ln1bi = np.floor((np.float32(ln1_b) / np.float32(ln1_w)) / sfp).astype(np.float32)[None, :]
    ln1sc = (sfp * np.float32(ln1_w)).astype(np.float32)[None, :]
    ln2bi = np.floor((np.float32(ln2_b) / np.float32(ln2_w)) / sfp).astype(np.float32)[None, :]
    ln2sc = (sfp * np.float32(ln2_w)).astype(np.float32)[None, :]
    bpw = (np.float32(b_proj) / np.maximum(wp_sf, 1e-37)).astype(np.float32)[None, :]

    scal = np.zeros((1, 8), np.float32)
    scal[0, 0] = _f32(1.0) / sf1
    scal[0, 1] = alpha_q
    scal[0, 2] = alpha_1
    scal[0, 3] = alpha_2

    nc = _get_module()
    from concourse.bass_utils import run_bass_kernel_spmd

    shared = dict(
        wqkvT=wqkvT, wprojT=wprojT, wfc1T=wfc1T, wfc2T=wfc2T,
        ln1bi=ln1bi, ln1sc=ln1sc, ln2bi=ln2bi, ln2sc=ln2sc,
        bpw=bpw, wsf=wp_sf.astype(np.float32)[None, :],
        bfc1=np.float32(b_fc1)[None, :], bfc2=np.float32(b_fc2)[None, :],
        scal=scal,
    )
    in_maps = []
    for c in range(NCORES):
        m = dict(shared)
        m["x1"] = np.ascontiguousarray(x_1[c * PER:(c + 1) * PER].reshape(M, C))
        in_maps.append(m)

    kw = {"trace": True} if _TRACE else {}
    res = run_bass_kernel_spmd(nc, in_maps, core_ids=list(range(NCORES)), **kw)
    _LAST_RESULT[0] = res

    stats = res.results[0]["stats"][0]
    max_av_int = float(stats[0])

    # Reference semantics: quant_act after attn@v computes sf = max|x|/127. If
    # that max is 0 (softmax collapsed -- the realistic case), x/sf = 0/0 = NaN
    # poisons every element downstream: output = all-NaN, sf = NaN.
    if max_av_int == 0.0 or not np.isfinite(max_av_int):
        return np.full((B, N, C), np.nan, dtype=np.float32), np.float32(np.nan)

    out = np.empty((B, N, C), np.float32)
    for c in range(NCORES):
        out[c * PER:(c + 1) * PER] = res.results[c]["out"].reshape(PER, N, C)
    return out, np.float32(stats[1])
